# revision 1
# baseline (speedup 1.0000x reference)
"""Trainium2 Bass kernel for nn_AnyNetRefinement (disparity refinement with SPN scan).

Data-parallel over batch: core b processes image b end-to-end (no collectives).
Pipeline per core:
  conv1..conv3 (3x3+BN+ReLU, bf16, row-stacked PE matmuls, DRAM-padded acts)
  conv4 -> raw gates G; convd (disp -> 8ch feature, f32)
  normalize gates (|G1|+|G2|+|G3|) writing A taps + w0 directly into
    scan-resident SBUF tiles
  SPN left-to-right scan over W=640 on VectorE (folded [128=(c,hblock), 26] state,
    3-tap multiply into a slot buffer + 4-slot tensor_reduce (slot 3 = w0,
    pre-staged by ScalarE) + stream_shuffle halos)
  convc (prop -> residual) + disp + relu -> out
"""

import numpy as np
import ml_dtypes

BF = ml_dtypes.bfloat16

H, W = 384, 640
HP, WP = 387, 642        # padded activation planes (+1 top/left, +2 bottom, +1 right)
X0S = (0, 320)
NX = 320                 # matmul free size (psum-bank safe)

_CACHE = {}
DO_MEMSET = True


# ---------------------------------------------------------------- host helpers
def _fold_bn(wt, g, b, m, v):
    s = g / np.sqrt(v + 1e-5)
    return (wt * s.reshape(-1, 1, 1, 1)).astype(np.float32), (b - m * s).astype(np.float32)


def _lhsT(wt, r_out, r_in, cin_g, npass):
    """lhsT [K=(cin_g,yi), npass, 3, M=(cout,r_out)]."""
    cout, cin = wt.shape[0], wt.shape[1]
    K = cin_g * r_in
    M = cout * r_out
    out = np.zeros((K, npass, 3, M), np.float32)
    for p in range(npass):
        for cg in range(cin_g):
            c = p * cin_g + cg
            if c >= cin:
                continue
            for dx in range(3):
                for yi in range(r_in):
                    k = cg * r_in + yi
                    for co in range(cout):
                        for yo in range(r_out):
                            dy = yi - yo
                            if 0 <= dy <= 2:
                                out[k, p, dx, co * r_out + yo] = wt[co, c, dy, dx]
    return out.reshape(K, npass * 3 * M)


def _pad_img(x, hp=HP, wp=WP):
    out = np.zeros((x.shape[0], hp, wp), BF)
    out[:, 1:1 + H, 1:1 + W] = x.astype(BF)
    return out


# ---------------------------------------------------------------- bass builder
def _build():
    import concourse.bass as bass
    import concourse.mybir as mybir
    from concourse import tile
    from concourse.vector_clock import ScopedClock

    f32 = mybir.dt.float32
    bf16 = mybir.dt.bfloat16
    ALU = mybir.AluOpType
    ACTF = mybir.ActivationFunctionType
    AX = mybir.AxisListType

    class TC(tile.TileContext):
        # this walrus build accepts only one sync-wait per Drain; split the
        # end-of-kernel waits across several drains.
        def _drain_and_barrier(self, tick_clock, wait_clock):
            nc = self.nc
            drain_inst = nc.sync.drain()
            wait_clock.add_sem_waits(drain_inst.ins, ScopedClock({None: tick_clock.global_clock}))
            waits = list(drain_inst.ins.sync_info.on_wait)
            if len(waits) > 1:
                drain_inst.ins.sync_info.on_wait = waits[:1]
                for i in range(1, len(waits)):
                    d2 = nc.sync.drain()
                    if d2.ins.sync_info is None:
                        d2.ins.sync_info = mybir.SyncInfo(on_wait=[waits[i]], on_update=[])
                    else:
                        d2.ins.sync_info.on_wait = [waits[i]]
            nc.all_engine_barrier()
            popped = nc._tile_sem_poison_stack.pop()
            assert popped is self._sem_poison
            nc.clear_and_free_semaphores(list(self.sems.allocated().values()))
            nc.all_engine_barrier()

    def dap(t, offset, dims):
        base = t if isinstance(t, bass.AP) else t[:]
        return bass.AP(base.tensor, base.offset + offset, [list(d) for d in dims])

    def sap(tile_ap, nparts, offset, dims, pstride=1):
        pstep = tile_ap.ap[0][0]
        return bass.AP(tile_ap.tensor, tile_ap.offset + offset,
                       [[pstep * pstride, nparts]] + [list(d) for d in dims])

    nc = bass.Bass("TRN2", num_swdge_queues=4)

    img = nc.declare_dram_parameter("img", [3, HP, WP], bf16, isOutput=False)
    dpad = nc.declare_dram_parameter("dpad", [1, HP, WP], bf16, isOutput=False)
    dispf = nc.declare_dram_parameter("dispf", [H, W], f32, isOutput=False)
    w1k = nc.declare_dram_parameter("w1k", [30, 3 * 128], bf16, isOutput=False)
    w2k = nc.declare_dram_parameter("w2k", [128, 3 * 96], bf16, isOutput=False)
    w3k = nc.declare_dram_parameter("w3k", [128, 3 * 96], bf16, isOutput=False)
    w4k = nc.declare_dram_parameter("w4k", [112, 3 * 120], bf16, isOutput=False)
    wdk = nc.declare_dram_parameter("wdk", [18, 3 * 128], bf16, isOutput=False)
    wck = nc.declare_dram_parameter("wck", [72, 2 * 3 * 16], bf16, isOutput=False)
    b1v = nc.declare_dram_parameter("b1v", [128, 1], f32, isOutput=False)
    b2v = nc.declare_dram_parameter("b2v", [96, 1], f32, isOutput=False)
    b3v = nc.declare_dram_parameter("b3v", [96, 1], f32, isOutput=False)
    outp = nc.declare_dram_parameter("out", [H, W], f32, isOutput=True)

    with TC(nc) as tc:
        with (tc.tile_pool(name="dram", bufs=1, space="DRAM") as dram,
              tc.tile_pool(name="wts", bufs=1) as pw):
            act1 = dram.tile([16, HP, WP], bf16, tag="act1")
            act2 = dram.tile([16, HP, WP], bf16, tag="act2")
            act3 = dram.tile([16, HP, WP], bf16, tag="act3")
            Gt = dram.tile([24, 385, W], bf16, tag="G")
            dfeat = dram.tile([8, H, W], f32, tag="dfeat")
            ppad = dram.tile([8, HP, WP], bf16, tag="ppad")

            # ---------------- weights/biases
            wtl = {}
            for nm, prm, kk, nm3 in (("w1", w1k, 30, 3 * 128), ("w2", w2k, 128, 3 * 96),
                                     ("w3", w3k, 128, 3 * 96), ("w4", w4k, 112, 3 * 120),
                                     ("wd", wdk, 18, 3 * 128), ("wc", wck, 72, 6 * 16)):
                t = pw.tile([kk, nm3], bf16, tag=f"{nm}t", name=f"{nm}t")
                nc.sync.dma_start(out=t[:], in_=prm[:])
                wtl[nm] = t
            b1t = pw.tile([128, 1], f32, tag="b1t")
            nc.sync.dma_start(out=b1t[:], in_=b1v[:])
            b2t = pw.tile([96, 1], f32, tag="b2t")
            nc.sync.dma_start(out=b2t[:], in_=b2v[:])
            b3t = pw.tile([96, 1], f32, tag="b3t")
            nc.sync.dma_start(out=b3t[:], in_=b3v[:])

            # ---------------- zero row-borders of padded internal buffers
            zt = pw.tile([128, 2 * WP], bf16, tag="zt")
            nc.vector.memset(zt[:], 0.0)
            ztv = zt[:]
            for buf, cc in ((act1, 16), (act2, 16), (act3, 16), (ppad, 8)):
                nc.sync.dma_start(out=dap(buf, 0, [(HP * WP, cc), (1, WP)]),
                                  in_=sap(ztv, cc, 0, [(1, WP)]))
                nc.sync.dma_start(out=dap(buf, 385 * WP, [(HP * WP, cc), (1, 2 * WP)]),
                                  in_=sap(ztv, cc, 0, [(1, 2 * WP)]))
                for col in (0, WP - 1):
                    nc.gpsimd.dma_start(out=dap(buf, col, [(HP * WP, cc), (WP, HP), (1, 1)]),
                                        in_=sap(ztv, cc, 0, [(1, HP), (0, 1)]))

            # ---------------- generic conv
            rhs_eng = [0]

            def conv(prhs, pout, ppsum, src, dst, wt, wK, wM, btile, cin_g, npass,
                     cout, r, rin, S, GS,
                     relu, dst_plane, dst_w, dst_pad, out_f32=False, disp_add=False):
                K = cin_g * rin
                assert K == wK
                M = cout * r
                assert M == wM
                wv = wt[:]
                ow = WP if dst_pad else W
                g0 = 0
                while g0 < S:
                    nsl = min(GS, S - g0)
                    y0 = r * g0
                    rhss = []
                    for p_ in range(npass):
                        rt = prhs.tile([K, GS, WP], bf16, tag="rhs", name="rhs")
                        for sl in range(nsl):
                            eng = nc.sync
                            rhs_eng[0] += 1
                            eng.dma_start(
                                out=rt[:, sl, :],
                                in_=dap(src, p_ * cin_g * HP * WP + (y0 + sl * r) * WP,
                                        [(HP * WP, cin_g), (WP, rin), (1, WP)]))
                        rhss.append(rt)
                    ps = []
                    for sl in range(nsl):
                        pstile = ppsum.tile([128, 2 * NX], f32, tag="ps", name="ps")
                        ps.append(pstile)
                    for x0, nxw in ((0, 512), (512, 128)):
                        for p_ in range(npass):
                            for dx in range(3):
                                for sl in range(nsl):
                                    nc.tensor.matmul(
                                        ps[sl][:M, x0:x0 + nxw],
                                        sap(wv, K, (p_ * 3 + dx) * M, [(1, M)]),
                                        rhss[p_][:, sl, x0 + dx:x0 + dx + nxw],
                                        start=(p_ == 0 and dx == 0),
                                        stop=(p_ == npass - 1 and dx == 2))
                    ot = pout.tile([M, GS, 2 * NX], f32 if out_f32 else bf16, tag="cout", name="cout")
                    for sl in range(nsl):
                        p = ps[sl][:M, :]
                        o = ot[:, sl, :]
                        if disp_add:
                            dt_ = pout.tile([16, 2 * NX], f32, tag="dtile", name="dtile")
                            nc.sync.dma_start(
                                out=dt_[:],
                                in_=dap(dispf, (y0 + sl * r) * W, [(W, 16), (1, 2 * NX)]))
                            tmp = pout.tile([16, 2 * NX], f32, tag="ctmp", name="ctmp")
                            nc.vector.tensor_tensor(out=tmp[:], in0=p, in1=dt_[:], op=ALU.add)
                            nc.vector.tensor_scalar(o, tmp[:], 0.0, None, ALU.max)
                        elif relu:
                            nc.vector.tensor_scalar(o, p, btile[:M, :], 0.0, ALU.add, ALU.max)
                        else:
                            nc.vector.tensor_copy(o, p)
                    for sl in range(nsl):
                        nc.scalar.dma_start(
                            out=dap(dst, ((1 if dst_pad else 0) + y0 + sl * r) * dst_w
                                    + (1 if dst_pad else 0),
                                    [(dst_plane, cout), (dst_w, r), (1, 2 * NX)]),
                            in_=ot[:, sl, :])
                    g0 += nsl

            with (tc.tile_pool(name="rhs", bufs=16) as prhs,
                  tc.tile_pool(name="cout", bufs=16) as pout,
                  tc.tile_pool(name="psum", bufs=4, space="PSUM") as ppsum):
                P3 = (prhs, pout, ppsum)
                conv(*P3, img, act1, wtl["w1"], 30, 128, b1t, 3, 1, 16, 8, 10, 48, 1,
                     True, HP * WP, WP, True)
                conv(*P3, act1, act2, wtl["w2"], 128, 96, b2t, 16, 1, 16, 6, 8, 64, 1,
                     True, HP * WP, WP, True)
                conv(*P3, act2, act3, wtl["w3"], 128, 96, b3t, 16, 1, 16, 6, 8, 64, 1,
                     True, HP * WP, WP, True)
                conv(*P3, act3, Gt, wtl["w4"], 112, 120, None, 16, 1, 24, 5, 7, 77, 1,
                     False, 385 * W, W, False)
                conv(*P3, dpad, dfeat, wtl["wd"], 18, 128, None, 1, 1, 8, 16, 18, 24, 1,
                     False, H * W, W, False, out_f32=True)

            # ---------------- scan-resident gate/w0 tiles
            with tc.tile_pool(name="scanbig", bufs=1) as pbig:
                gw = pbig.tile([128, 3, 24, W], bf16, tag="gw")
                gwv = gw[:]
                w0w = pbig.tile([128, 24, W], bf16, tag="w0w")
                w0v = w0w[:]
                nc.vector.memset(gwv, 0.0)
                nc.vector.memset(w0v, 0.0)

                # ---------------- gate normalization (direct into gw/w0w)
                NCH = 16
                CH = 15360 // NCH
                GP = 385 * W
                with (tc.tile_pool(name="norm3", bufs=4) as pn3,
                      tc.tile_pool(name="norm1", bufs=2) as pn1):
                    gpair = [None]
                    for k in range(NCH):
                        if k % 2 == 0:
                            gp = []
                            for tap in range(3):
                                g2 = pn3.tile([128, 2 * CH], bf16, tag="gld", name="gld")
                                nc.sync.dma_start(
                                    out=g2[:],
                                    in_=dap(Gt, tap * 8 * GP + k * CH,
                                            [(GP, 8), (24 * W, 16), (1, 2 * CH)]))
                                gp.append(g2)
                            gpair[0] = gp
                        gts = [gpair[0][tap][:, (k % 2) * CH:(k % 2) * CH + CH]
                               for tap in range(3)]
                        ab = []
                        for tap in range(3):
                            a = pn3.tile([128, CH], bf16, tag="gabs", name="gabs")
                            nc.scalar.activation(a[:], gts[tap][:], ACTF.Abs)
                            ab.append(a)
                        s12 = pn1.tile([128, CH], bf16, tag="s12")
                        nc.gpsimd.tensor_tensor(out=s12[:], in0=ab[0][:], in1=ab[1][:], op=ALU.add)
                        sf = pn1.tile([128, CH], f32, tag="sf")
                        nc.vector.scalar_tensor_tensor(out=sf[:], in0=ab[2][:], scalar=1e-8,
                                                       in1=s12[:], op0=ALU.add, op1=ALU.add)
                        lns = pn1.tile([128, CH], f32, tag="lns")
                        nc.scalar.activation(lns[:], sf[:], ACTF.Ln)
                        rs = pn1.tile([128, CH], f32, tag="rs")
                        nc.scalar.activation(rs[:], lns[:], ACTF.Exp, scale=-1.0)
                        gsl = []
                        for tap in range(3):
                            o = sap(gwv, 128, tap * 24 * W + k * CH, [(1, CH)])
                            nc.vector.tensor_tensor(out=o, in0=gts[tap][:], in1=rs[:], op=ALU.mult)
                            gsl.append(o)
                        a12 = pn1.tile([128, CH], bf16, tag="a12")
                        nc.gpsimd.tensor_tensor(out=a12[:], in0=gsl[0], in1=gsl[1], op=ALU.add)
                        asum = pn1.tile([128, CH], bf16, tag="asum")
                        nc.gpsimd.tensor_tensor(out=asum[:], in0=a12[:], in1=gsl[2], op=ALU.add)
                        t2 = pn1.tile([128, CH], bf16, tag="t2")
                        nc.vector.tensor_scalar(t2[:], asum[:], -1.0, 1.0, ALU.mult, ALU.add)
                        df = pn1.tile([128, CH], f32, tag="dfl")
                        nc.sync.dma_start(out=df[:],
                                            in_=dap(dfeat, k * CH, [(H * W, 8), (24 * W, 16), (1, CH)]))
                        nc.vector.tensor_tensor(out=sap(w0v, 128, k * CH, [(1, CH)]),
                                                in0=t2[:], in1=df[:], op=ALU.mult)
                    # zero edge gates: tap0 (up) at row 0 of hb=0; tap2 (dn) at row 23 of hb=15
                    pstep = gwv.ap[0][0]
                    for c_ in range(8):
                        nc.sync.dma_start(
                            out=bass.AP(gwv.tensor, gwv.offset + (16 * c_) * pstep,
                                        [[pstep, 1], [1, W]]),
                            in_=sap(ztv, 1, 0, [(1, W)]))
                        nc.sync.dma_start(
                            out=bass.AP(gwv.tensor,
                                        gwv.offset + (16 * c_ + 15) * pstep + 2 * 24 * W + 23 * W,
                                        [[pstep, 1], [1, W]]),
                            in_=sap(ztv, 1, 0, [(1, W)]))

                # ---------------- SPN scan
                mask_up = [(i - 1) % 32 for i in range(32)]
                mask_dn = [(i + 1) % 32 for i in range(32)]
                TB = 32  # w0-staging chunk
                with tc.tile_pool(name="scansm", bufs=1) as psm:
                    # pf: prop buffer doubling as scan state (bf16).
                    # rows: 0 = up-halo, 1..24 = block rows, 25 = dn-halo.
                    # col 1+t holds h_t; col 0 = zero initial state.
                    pf = psm.tile([128, 26, WP], bf16, tag="pf")
                    pfv = pf[:]
                    nc.vector.memset(pfv, 0.0)
                    prw = psm.tile([128, 2, TB, 24, 4], f32, tag="prw")
                    prv = prw[:]
                    for t in range(W):
                        j = t % TB
                        bi = (t // TB) % 2
                        if j == 0:
                            nc.scalar.copy(
                                sap(prv, 128, bi * (TB * 96) + 3, [(4, 24), (96, TB)]),
                                sap(w0v, 128, t, [(W, 24), (1, TB)]))
                        base = bi * (TB * 96) + j * 96
                        taps = sap(pfv, 128, t, [(WP, 24), (WP, 3)])
                        g_t = sap(gwv, 128, t, [(W, 24), (24 * W, 3)])
                        nc.vector.tensor_tensor(out=sap(prv, 128, base, [(4, 24), (1, 3)]),
                                                in0=g_t, in1=taps, op=ALU.mult)
                        with nc.allow_low_precision(reason="bf16 scan state, validated"):
                            nc.vector.tensor_reduce(out=sap(pfv, 128, WP + 1 + t, [(WP, 24)]),
                                                    in_=sap(prv, 128, base, [(4, 24), (1, 4)]),
                                                    axis=AX.X, op=ALU.add)
                        nc.vector.stream_shuffle(out=sap(pfv, 128, 1 + t, [(1, 1)]),
                                                 in_=sap(pfv, 128, 24 * WP + 1 + t, [(1, 1)]),
                                                 mask=mask_up)
                        nc.vector.stream_shuffle(out=sap(pfv, 128, 25 * WP + 1 + t, [(1, 1)]),
                                                 in_=sap(pfv, 128, WP + 1 + t, [(1, 1)]),
                                                 mask=mask_dn)
                    # export prop rows 1..24 -> ppad rows 1..384, full width
                    nc.scalar.dma_start(
                        out=dap(ppad, WP, [(HP * WP, 8), (24 * WP, 16), (1, 24 * WP)]),
                        in_=sap(pfv, 128, WP, [(1, 24 * WP)]))

            # ---------------- convc: ppad -> out (+disp, relu)
            with (tc.tile_pool(name="rhsc", bufs=16) as prhs2,
                  tc.tile_pool(name="coutc", bufs=16) as pout2,
                  tc.tile_pool(name="psumc", bufs=4, space="PSUM") as ppsum2):
                conv(prhs2, pout2, ppsum2, ppad, outp, wtl["wc"], 72, 16, None, 4, 2,
                     1, 16, 18, 24, 1,
                     False, H * W, W, False, out_f32=True, disp_add=True)

    # Engine-sem update thinning: Tile increments each engine's sem on every
    # op, but only values that some wait references matter. Dropping the rest
    # (and renumbering waits to the kept-update count at the same producer op)
    # is semantically exact and shrinks the sem-update backlog.
    ENG_SEMS = ("DVE_44", "PE_44", "Activation_44", "Pool_44", "SP_44")
    insts_all = []
    for fn in nc.m.functions:
        for bb in fn.blocks:
            insts_all.extend(bb.instructions)
    waited = {sm: set() for sm in ENG_SEMS}
    for inst in insts_all:
        si = inst.sync_info
        if si and si.on_wait:
            for wt_ in si.on_wait:
                if wt_.ant_name in waited:
                    waited[wt_.ant_name].add(wt_.wait_value)
    cum = {sm: 0 for sm in ENG_SEMS}
    newcum = {sm: 0 for sm in ENG_SEMS}
    remap = {sm: {} for sm in ENG_SEMS}
    for inst in insts_all:
        si = inst.sync_info
        if not si:
            continue
        ups = si.on_update
        if ups:
            keep = []
            for u in ups:
                sm = u.ant_name
                if sm in cum:
                    cum[sm] += 1
                    if cum[sm] in waited[sm]:
                        newcum[sm] += 1
                        remap[sm][cum[sm]] = newcum[sm]
                        keep.append(u)
                    # else: drop this update
                else:
                    keep.append(u)
            if len(keep) != len(ups):
                si.on_update = keep
    for inst in insts_all:
        si = inst.sync_info
        if si and si.on_wait:
            ws = list(si.on_wait)
            ch = False
            for i_, wt_ in enumerate(ws):
                if wt_.ant_name in remap and wt_.wait_value in remap[wt_.ant_name]:
                    nv = remap[wt_.ant_name][wt_.wait_value]
                    if nv != wt_.wait_value:
                        wt_.wait_value = nv
                        ch = True
            if ch:
                si.on_wait = ws

    # walrus on this stack accepts at most one sync wait per instruction:
    # spill excess waits onto same-engine NOPs injected just before.
    nwn = [0]
    for fn in nc.m.functions:
        for bb in fn.blocks:
            il = bb.instructions
            i = 0
            while i < len(il):
                inst = il[i]
                si = inst.sync_info
                if si is not None and si.on_wait and len(si.on_wait) > 1:
                    waits = list(si.on_wait)
                    si.on_wait = waits[-1:]
                    for j in range(len(waits) - 1):
                        nwn[0] += 1
                        nop = mybir.InstNoOp(name=f"WS-{nwn[0]}", ins=[], outs=[])
                        nop.engine = inst.engine
                        nop.sync_info = mybir.SyncInfo(on_wait=[waits[j]], on_update=[])
                        nc.register_instruction(nop, overwrite=True)
                        il.insert(i, nop)
                        i += 1
                i += 1

    return nc


def _prep_inputs(inputs):
    w1, b1 = _fold_bn(inputs['w1'], inputs['bn1_g'], inputs['bn1_b'], inputs['bn1_m'], inputs['bn1_v'])
    w2, b2 = _fold_bn(inputs['w2'], inputs['bn2_g'], inputs['bn2_b'], inputs['bn2_m'], inputs['bn2_v'])
    w3, b3 = _fold_bn(inputs['w3'], inputs['bn3_g'], inputs['bn3_b'], inputs['bn3_m'], inputs['bn3_v'])

    w1k = _lhsT(w1, 8, 10, 3, 1).astype(BF)                       # [30, 384]
    w2k = _lhsT(w2, 6, 8, 16, 1).astype(BF)                       # [128, 288]
    w3k = _lhsT(w3, 6, 8, 16, 1).astype(BF)
    w4k = _lhsT(inputs['w4'].astype(np.float32), 5, 7, 16, 1).astype(BF)   # [112, 360]
    wdk = _lhsT(inputs['wd'].astype(np.float32), 16, 18, 1, 1).astype(BF)  # [18, 384]
    wck = _lhsT(inputs['wc'].astype(np.float32), 16, 18, 4, 2).astype(BF)  # [72, 96]

    b1r = np.repeat(b1, 8).reshape(128, 1).astype(np.float32)
    b2r = np.repeat(b2, 6).reshape(96, 1).astype(np.float32)
    b3r = np.repeat(b3, 6).reshape(96, 1).astype(np.float32)

    maps = []
    for b in range(8):
        maps.append({
            "img": _pad_img(inputs['leftImage'][b]),
            "dpad": _pad_img(inputs['disp'][b]),
            "dispf": inputs['disp'][b, 0].astype(np.float32),
            "w1k": w1k, "w2k": w2k, "w3k": w3k, "w4k": w4k, "wdk": wdk, "wck": wck,
            "b1v": b1r, "b2v": b2r, "b3v": b3r,
        })
    return maps


def kernel(**inputs):
    from concourse.bass_utils import run_bass_kernel_spmd

    if "nc" not in _CACHE:
        _CACHE["nc"] = _build()
    nc = _CACHE["nc"]
    maps = _prep_inputs(inputs)
    res = run_bass_kernel_spmd(nc, maps, core_ids=list(range(8)))
    out = np.stack([res.results[i]["out"] for i in range(8)])[:, None].astype(np.float32)
    return out



# revision 38
# speedup vs baseline: 3.2512x; 3.2512x over previous
"""Trainium2 Bass kernel for nn_AnyNetRefinement (disparity refinement with SPN scan).

Data-parallel over batch: core b processes image b end-to-end (no collectives).

v2: all DRAM round-trips use row-interleaved layouts ([row, channel, col]) with
matmul M-order (row, channel), so every conv load/store DMA collapses to a
2-dim AP [(WP, M), (1, cols)] (single-descriptor-chain cost). Epilogues on
gpsimd, writes on scalar queue, loads on sync queue. Gate normalization is
chunked per output row and balanced across Act/Pool/DVE. The SPN scan is the
folded [128=(c,hblock), 26] bf16 state machine (3-tap multiply + 4-slot
reduce + halo shuffles). Prop is exported to a channel-planar DRAM buffer in
24 merged per-block-row DMAs and convc consumes it directly.
"""

import numpy as np
import ml_dtypes

BF = ml_dtypes.bfloat16

H, W = 384, 640
HP, WP = 387, 642        # padded rows (+1 top, +2 bottom), cols (+1 left, +1 right)
NX = 320                 # half free width (psum-bank safe: 512+128 split)

_CACHE = {}


# ---------------------------------------------------------------- host helpers
def _fold_bn(wt, g, b, m, v):
    s = g / np.sqrt(v + 1e-5)
    return (wt * s.reshape(-1, 1, 1, 1)).astype(np.float32), (b - m * s).astype(np.float32)


def _lhsT(wt, r_out, r_in, cin_g, npass):
    """lhsT [K=(cin_g,yi), npass, 3, M=(yo,cout)] -- M row-major."""
    cout, cin = wt.shape[0], wt.shape[1]
    K = cin_g * r_in
    M = cout * r_out
    out = np.zeros((K, npass, 3, M), np.float32)
    for p in range(npass):
        for cg in range(cin_g):
            c = p * cin_g + cg
            if c >= cin:
                continue
            for dx in range(3):
                for yi in range(r_in):
                    k = cg * r_in + yi
                    for co in range(cout):
                        for yo in range(r_out):
                            dy = yi - yo
                            if 0 <= dy <= 2:
                                out[k, p, dx, yo * cout + co] = wt[co, c, dy, dx]
    return out.reshape(K, npass * 3 * M)


def _pad_rowi(x):
    """[C, H, W] -> row-interleaved padded [HP, C, WP] bf16."""
    c = x.shape[0]
    out = np.zeros((HP, c, WP), BF)
    out[1:1 + H, :, 1:1 + W] = np.moveaxis(x, 0, 1).astype(BF)
    return out


# ---------------------------------------------------------------- bass builder
def _build():
    import concourse.bass as bass
    import concourse.mybir as mybir
    from concourse import tile
    from concourse.vector_clock import ScopedClock

    f32 = mybir.dt.float32
    bf16 = mybir.dt.bfloat16
    ALU = mybir.AluOpType
    ACTF = mybir.ActivationFunctionType
    AX = mybir.AxisListType

    class TC(tile.TileContext):
        # this walrus build accepts only one sync-wait per Drain; split the
        # end-of-kernel waits across several drains.
        def _drain_and_barrier(self, tick_clock, wait_clock):
            nc = self.nc
            drain_inst = nc.sync.drain()
            wait_clock.add_sem_waits(drain_inst.ins, ScopedClock({None: tick_clock.global_clock}))
            waits = list(drain_inst.ins.sync_info.on_wait)
            if len(waits) > 1:
                drain_inst.ins.sync_info.on_wait = waits[:1]
                for i in range(1, len(waits)):
                    d2 = nc.sync.drain()
                    if d2.ins.sync_info is None:
                        d2.ins.sync_info = mybir.SyncInfo(on_wait=[waits[i]], on_update=[])
                    else:
                        d2.ins.sync_info.on_wait = [waits[i]]
            nc.all_engine_barrier()
            popped = nc._tile_sem_poison_stack.pop()
            assert popped is self._sem_poison
            nc.clear_and_free_semaphores(list(self.sems.allocated().values()))
            nc.all_engine_barrier()

    def dap(t, offset, dims):
        base = t if isinstance(t, bass.AP) else t[:]
        return bass.AP(base.tensor, base.offset + offset, [list(d) for d in dims])

    def sap(tile_ap, nparts, offset, dims, pstride=1):
        pstep = tile_ap.ap[0][0]
        return bass.AP(tile_ap.tensor, tile_ap.offset + offset,
                       [[pstep * pstride, nparts]] + [list(d) for d in dims])

    nc = bass.Bass("TRN2", num_swdge_queues=4)

    img = nc.declare_dram_parameter("img", [HP, 3, WP], bf16, isOutput=False)
    dpad = nc.declare_dram_parameter("dpad", [HP, 1, WP], bf16, isOutput=False)
    dispb = nc.declare_dram_parameter("dispb", [H, W], bf16, isOutput=False)
    w1k = nc.declare_dram_parameter("w1k", [30, 3 * 128], bf16, isOutput=False)
    w2k = nc.declare_dram_parameter("w2k", [128, 3 * 96], bf16, isOutput=False)
    w3k = nc.declare_dram_parameter("w3k", [128, 3 * 96], bf16, isOutput=False)
    w4k = nc.declare_dram_parameter("w4k", [112, 3 * 120], bf16, isOutput=False)
    wdk = nc.declare_dram_parameter("wdk", [18, 3 * 128], bf16, isOutput=False)
    wck = nc.declare_dram_parameter("wck", [72, 2 * 3 * 16], bf16, isOutput=False)
    b1v = nc.declare_dram_parameter("b1v", [128, 1], f32, isOutput=False)
    b2v = nc.declare_dram_parameter("b2v", [96, 1], f32, isOutput=False)
    b3v = nc.declare_dram_parameter("b3v", [96, 1], f32, isOutput=False)
    idv = nc.declare_dram_parameter("idv", [16, 16], bf16, isOutput=False)
    outp = nc.declare_dram_parameter("out", [H, W], f32, isOutput=True)

    with TC(nc) as tc:
        with (tc.tile_pool(name="dram", bufs=1, space="DRAM") as dram,
              tc.tile_pool(name="wts", bufs=1) as pw):
            act1 = dram.tile([HP, 16, WP], bf16, tag="act1")
            act2 = dram.tile([HP, 16, WP], bf16, tag="act2")
            act3 = dram.tile([HP, 16, WP], bf16, tag="act3")
            Gt = dram.tile([385, 24, W], bf16, tag="G")
            dfeat = dram.tile([H, 8, W], bf16, tag="dfeat")
            prop = dram.tile([8, H, WP], bf16, tag="prop")

            # ---------------- weights/biases
            wtl = {}
            for nm, prm, kk, nm3 in (("w1", w1k, 30, 3 * 128), ("w2", w2k, 128, 3 * 96),
                                     ("w3", w3k, 128, 3 * 96), ("w4", w4k, 112, 3 * 120),
                                     ("wd", wdk, 18, 3 * 128), ("wc", wck, 72, 6 * 16)):
                t = pw.tile([kk, nm3], bf16, tag=f"{nm}t", name=f"{nm}t")
                nc.sync.dma_start(out=t[:], in_=prm[:])
                wtl[nm] = t
            b1t = pw.tile([128, 1], f32, tag="b1t")
            nc.sync.dma_start(out=b1t[:], in_=b1v[:])
            b2t = pw.tile([96, 1], f32, tag="b2t")
            nc.sync.dma_start(out=b2t[:], in_=b2v[:])
            b3t = pw.tile([96, 1], f32, tag="b3t")
            nc.sync.dma_start(out=b3t[:], in_=b3v[:])
            idt = pw.tile([16, 16], bf16, tag="idt")
            nc.sync.dma_start(out=idt[:], in_=idv[:])

            # ---------------- zero borders of padded DRAM buffers
            zt = pw.tile([128, WP], bf16, tag="zt")
            nc.vector.memset(zt[:], 0.0)
            ztv = zt[:]
            for buf in (act1, act2, act3):
                # row 0 (16ch) and rows 385,386 (32 rows*ch), full padded width
                nc.gpsimd.dma_start(out=dap(buf, 0, [(WP, 16), (1, WP)]),
                                    in_=sap(ztv, 16, 0, [(1, WP)]))
                nc.gpsimd.dma_start(out=dap(buf, 385 * 16 * WP, [(WP, 32), (1, WP)]),
                                    in_=sap(ztv, 32, 0, [(1, WP)]))
                # cols 0 and WP-1, rows 1..384 all ch (merged (row,ch) stride WP)
                for col in (0, WP - 1):
                    nc.gpsimd.dma_start(
                        out=dap(buf, 16 * WP + col, [(WP, 16 * H), (1, 1)]),
                        in_=sap(ztv, 128, 0, [(0, 48), (1, 1)]))
            # prop: cols 0/641, all rows*channels (merged stride WP)
            for col in (0, WP - 1):
                nc.gpsimd.dma_start(
                    out=dap(prop, col, [(WP, 8 * H), (1, 1)]),
                    in_=sap(ztv, 128, 0, [(0, 24), (1, 1)]))

            # ---------------- generic conv (row-group pipelined)
            def conv(prhs, ppsum, pout, wt, K, M, npass, S, load, dst_ap, epi,
                     odt, st):
                wv = wt[:]
                for g in range(S):
                    rhss = []
                    for p_ in range(npass):
                        rt = prhs.tile([K, WP], bf16, tag="rhs", name="rhs")
                        load(rt, p_, g)
                        rhss.append(rt)
                    ps = ppsum.tile([128, 2 * NX], f32, tag="ps", name="ps")
                    for x0, nxw in ((0, 512), (512, 128)):
                        for p_ in range(npass):
                            for dx in range(3):
                                nc.tensor.matmul(
                                    ps[:M, x0:x0 + nxw],
                                    sap(wv, K, (p_ * 3 + dx) * M, [(1, M)]),
                                    rhss[p_][:, x0 + dx:x0 + dx + nxw],
                                    start=(p_ == 0 and dx == 0),
                                    stop=(p_ == npass - 1 and dx == 2))
                    ot = pout.tile([M, 2 * NX], odt, tag="cout", name="cout")
                    epi(ps, ot, g)
                    weng = st if g % 2 == 0 else nc.gpsimd
                    weng.dma_start(out=dst_ap(g), in_=ot[:])

            with (tc.tile_pool(name="rhs", bufs=12) as prhs,
                  tc.tile_pool(name="cout", bufs=12) as pout,
                  tc.tile_pool(name="psum", bufs=4, space="PSUM") as ppsum):
                # NOTE: gpsimd cannot touch PSUM (BIR verifier) -- epilogues
                # that read PSUM live on DVE (tensor_scalar) or Act (activation).
                def bias_relu(bt, M):
                    def epi(ps, ot, g):
                        nc.vector.tensor_scalar(ot[:], ps[:M, :], bt[:M, :], 0.0,
                                                ALU.add, ALU.max)
                    return epi

                def copy_epi(M):
                    def epi(ps, ot, g):
                        nc.scalar.activation(ot[:], ps[:M, :], ACTF.Copy)
                    return epi

                def simple_load(src, cg_n, rin, r, cpitch):
                    def load(rt, p_, g):
                        nc.sync.dma_start(
                            out=rt[:],
                            in_=dap(src, (r * g) * cpitch * WP,
                                    [(WP, cg_n), (cpitch * WP, rin), (1, WP)])
                            if cg_n > 1 else
                            dap(src, (r * g) * WP, [(WP, rin), (1, WP)]))
                    return load

                # conv1: img -> act1  (K=30=(3cg,10yi), M=128=(8yo,16c))
                conv(prhs, ppsum, pout, wtl["w1"], 30, 128, 1, 48,
                     simple_load(img, 3, 10, 8, 3),
                     lambda g: dap(act1, (1 + 8 * g) * 16 * WP + 1,
                                   [(WP, 128), (1, 640)]),
                     bias_relu(b1t, 128), bf16, nc.scalar)
                # conv2: act1 -> act2  (K=128=(16cg,8yi), M=96=(6yo,16c))
                conv(prhs, ppsum, pout, wtl["w2"], 128, 96, 1, 64,
                     simple_load(act1, 16, 8, 6, 16),
                     lambda g: dap(act2, (1 + 6 * g) * 16 * WP + 1,
                                   [(WP, 96), (1, 640)]),
                     bias_relu(b2t, 96), bf16, nc.scalar)
                # conv3: act2 -> act3
                conv(prhs, ppsum, pout, wtl["w3"], 128, 96, 1, 64,
                     simple_load(act2, 16, 8, 6, 16),
                     lambda g: dap(act3, (1 + 6 * g) * 16 * WP + 1,
                                   [(WP, 96), (1, 640)]),
                     bias_relu(b3t, 96), bf16, nc.scalar)
                # convd: dpad -> dfeat (K=18, M=128=(16yo,8c)) bf16 out
                conv(prhs, ppsum, pout, wtl["wd"], 18, 128, 1, 24,
                     simple_load(dpad, 1, 18, 16, 1),
                     lambda g: dap(dfeat, (16 * g) * 8 * W, [(W, 128), (1, 640)]),
                     copy_epi(128), bf16, nc.scalar)
                # conv4: act3 -> Gt (K=112=(16cg,7yi), M=120=(5yo,24c))
                conv(prhs, ppsum, pout, wtl["w4"], 112, 120, 1, 77,
                     simple_load(act3, 16, 7, 5, 16),
                     lambda g: dap(Gt, (5 * g) * 24 * W, [(W, 120), (1, 640)]),
                     copy_epi(120), bf16, nc.scalar)

            # ---------------- scan-resident gate/w0 tiles + normalization
            with tc.tile_pool(name="scanbig", bufs=1) as pbig:
                # gw tap-fastest [24 rows, W cols, 3 taps]; pf column-major
                # [WP cols, 26 rows] -- every scan AP is then innermost-
                # contiguous, which unlocks the DVE 2x bf16 mode for the
                # per-step 3-tap multiply.
                gw = pbig.tile([128, 24, W, 3], bf16, tag="gw")
                gwv = gw[:]
                w0w = pbig.tile([128, 24, W], bf16, tag="w0w")
                w0v = w0w[:]
                TB = 16  # w0-staging chunk
                pf = pbig.tile([128, WP, 26], bf16, tag="pf")
                pfv = pf[:]
                # only col 0 (initial state + first-step halos) must be zero;
                # every other cell is written before it is read.
                nc.vector.memset(sap(pfv, 128, 0, [(1, 26)]), 0.0)
                prw = pbig.tile([128, 4, TB, 24, 4], bf16, tag="prw")
                prv = prw[:]

                # normalization: per output row r (24 rows per h-block), partitions
                # are (c:8, hb:16); Gt row-interleaved [385,24,W] gives 3-dim APs.
                with (tc.tile_pool(name="norm3", bufs=7) as pn3,
                      tc.tile_pool(name="norm1", bufs=3) as pn1,
                      tc.tile_pool(name="normf", bufs=3) as pnf):
                    for r in range(24):
                        gl = []
                        for tap in range(3):
                            g2 = pn3.tile([128, W], bf16, tag="gld", name="gld")
                            nc.sync.dma_start(
                                out=g2[:],
                                in_=dap(Gt, r * 24 * W + tap * 8 * W,
                                        [(W, 8), (24 * 24 * W, 16), (1, W)]))
                            gl.append(g2)
                        df = pn1.tile([128, W], bf16, tag="dfl", name="dfl")
                        nc.sync.dma_start(
                            out=df[:],
                            in_=dap(dfeat, r * 8 * W, [(W, 8), (24 * 8 * W, 16), (1, W)]))
                        ab = []
                        for tap in range(3):
                            a = pn3.tile([128, W], bf16, tag="gabs", name="gabs")
                            nc.scalar.activation(a[:], gl[tap][:], ACTF.Abs)
                            ab.append(a)
                        s12 = pn1.tile([128, W], bf16, tag="s12")
                        nc.vector.scalar_tensor_tensor(out=s12[:], in0=ab[0][:],
                                                       scalar=1e-8, in1=ab[1][:],
                                                       op0=ALU.add, op1=ALU.add)
                        sf = pnf.tile([128, W], f32, tag="sf")
                        nc.gpsimd.tensor_tensor(out=sf[:], in0=ab[2][:], in1=s12[:],
                                                op=ALU.add)
                        nc.vector.reciprocal(sf[:], sf[:])
                        for tap, eng in zip(range(3), (nc.gpsimd, nc.gpsimd, nc.vector)):
                            o = sap(gwv, 128, r * (W * 3) + tap, [(3, W)])
                            eng.tensor_tensor(out=o, in0=gl[tap][:], in1=sf[:],
                                              op=ALU.mult)
                        # t2 must come from the ROUNDED bf16 gates so the scan
                        # coefficients sum to exactly 1 (else drift accumulates
                        # over the 640-step scan).
                        gs0 = sap(gwv, 128, r * (W * 3) + 0, [(3, W)])
                        gs1 = sap(gwv, 128, r * (W * 3) + 1, [(3, W)])
                        gs2 = sap(gwv, 128, r * (W * 3) + 2, [(3, W)])
                        a12 = pn1.tile([128, W], bf16, tag="a12")
                        nc.gpsimd.tensor_tensor(out=a12[:], in0=gs0, in1=gs1,
                                                op=ALU.add)
                        nc.vector.tensor_tensor(out=a12[:], in0=a12[:], in1=gs2,
                                                op=ALU.add)
                        nc.scalar.activation(a12[:], a12[:], ACTF.Identity,
                                             bias=1.0, scale=-1.0)
                        nc.gpsimd.tensor_tensor(out=sap(w0v, 128, r * W, [(1, W)]),
                                                in0=a12[:], in1=df[:], op=ALU.mult)
                    # zero edge gates: tap0 (up) row 0 of hb=0; tap2 (dn) row 23 of hb=15
                    pstep = gwv.ap[0][0]
                    nc.sync.dma_start(
                        out=bass.AP(gwv.tensor, gwv.offset,
                                    [[16 * pstep, 8], [3, W], [1, 1]]),
                        in_=sap(ztv, 8, 0, [(1, W), (0, 1)]))
                    for c_ in range(8):
                        nc.sync.dma_start(
                            out=bass.AP(gwv.tensor,
                                        gwv.offset + (16 * c_ + 15) * pstep
                                        + 23 * (W * 3) + 2,
                                        [[pstep, 1], [3, W], [1, 1]]),
                            in_=sap(ztv, 1, 0, [(1, W), (0, 1)]))

                # ---------------- SPN scan (pf column-major: elem (x, r) at x*26+r)
                mask_up = [(i - 1) % 32 for i in range(32)]
                mask_dn = [(i + 1) % 32 for i in range(32)]
                for t in range(W):
                    j = t % TB
                    bi = (t // TB) % 4
                    if j == 0:
                        nc.scalar.copy(
                            sap(prv, 128, bi * (TB * 96) + 3, [(4, 24), (96, TB)]),
                            sap(w0v, 128, t, [(W, 24), (1, TB)]))
                    base = bi * (TB * 96) + j * 96
                    taps = sap(pfv, 128, t * 26, [(1, 24), (1, 3)])
                    g_t = sap(gwv, 128, t * 3, [(3 * W, 24), (1, 3)])
                    nc.vector.tensor_tensor(out=sap(prv, 128, base, [(4, 24), (1, 3)]),
                                            in0=g_t, in1=taps, op=ALU.mult)
                    with nc.allow_low_precision(reason="bf16 scan state, validated"):
                        nc.vector.tensor_reduce(out=sap(pfv, 128, (1 + t) * 26 + 1, [(1, 24)]),
                                                in_=sap(prv, 128, base, [(4, 24), (1, 4)]),
                                                axis=AX.X, op=ALU.add)
                    nc.vector.stream_shuffle(out=sap(pfv, 128, (1 + t) * 26, [(1, 1)]),
                                             in_=sap(pfv, 128, (1 + t) * 26 + 24, [(1, 1)]),
                                             mask=mask_up)
                    nc.vector.stream_shuffle(out=sap(pfv, 128, (1 + t) * 26 + 25, [(1, 1)]),
                                             in_=sap(pfv, 128, (1 + t) * 26 + 1, [(1, 1)]),
                                             mask=mask_dn)
                # ---------------- prop export + convc, split into column halves:
                # half A (out cols 0..319) depends only on scan steps <= 321,
                # so its exports/loads/matmuls/epilogue all run during the
                # scan's second half on the otherwise-idle engines.
                with (tc.tile_pool(name="rhsc", bufs=8) as prhs2,
                      tc.tile_pool(name="coutc", bufs=8) as pout2,
                      tc.tile_pool(name="psumc", bufs=4, space="PSUM") as ppsum2):
                    wcv = wtl["wc"][:]
                    NC2 = 322  # prop cols per rhs tile

                    def convc_half(xo, ex0, exn):
                        # export data cols ex0..ex0+exn-1 (prop col = x+1)
                        with nc.allow_non_contiguous_dma(reason="pf col-major"):
                            for r in range(24):
                                eng = nc.sync if r % 2 == 0 else nc.scalar
                                eng.dma_start(
                                    out=dap(prop, r * WP + 1 + ex0,
                                            [(24 * WP, 128), (1, exn)]),
                                    in_=sap(pfv, 128, (1 + ex0) * 26 + 1 + r,
                                            [(26, exn), (1, 1)]))
                        c0 = xo  # first prop col in rhs tiles (= x0-1+1)
                        for g in range(24):
                            rhss = []
                            for p_ in range(2):
                                ld = nc.sync if p_ == 0 else nc.scalar
                                rt = prhs2.tile([72, NC2], bf16, tag="rhs", name="rhsc")
                                rv = rt[:]
                                pstep = rv.ap[0][0]
                                if g == 0 or g == 23:
                                    zrow = 0 if g == 0 else 17
                                    drow0, prow0 = (1, 0) if g == 0 else (0, 367)
                                    for cg in range(4):
                                        nc.gpsimd.dma_start(
                                            out=bass.AP(rv.tensor,
                                                        rv.offset + (cg * 18 + zrow) * pstep,
                                                        [[pstep, 1], [1, NC2]]),
                                            in_=sap(ztv, 1, 0, [(1, NC2)]))
                                        ld.dma_start(
                                            out=bass.AP(rv.tensor,
                                                        rv.offset + (cg * 18 + drow0) * pstep,
                                                        [[pstep, 17], [1, NC2]]),
                                            in_=dap(prop,
                                                    (p_ * 4 + cg) * H * WP
                                                    + prow0 * WP + c0,
                                                    [(WP, 17), (1, NC2)]))
                                else:
                                    ld.dma_start(
                                        out=rv,
                                        in_=dap(prop,
                                                p_ * 4 * H * WP + (16 * g - 1) * WP + c0,
                                                [(H * WP, 4), (WP, 18), (1, NC2)]))
                                rhss.append(rt)
                            # disp residual enters PSUM via an identity matmul,
                            # so the epilogue is a single Act relu.
                            dt_ = pout2.tile([16, 320], bf16, tag="dtile", name="dtile")
                            nc.gpsimd.dma_start(
                                out=dt_[:],
                                in_=dap(dispb, (16 * g) * W + xo, [(W, 16), (1, 320)]))
                            ps = ppsum2.tile([16, 320], f32, tag="psc", name="psc")
                            for p_ in range(2):
                                for dx in range(3):
                                    nc.tensor.matmul(
                                        ps[:, :],
                                        sap(wcv, 72, (p_ * 3 + dx) * 16, [(1, 16)]),
                                        rhss[p_][:, dx:dx + 320],
                                        start=(p_ == 0 and dx == 0),
                                        stop=False)
                            nc.tensor.matmul(ps[:, :], idt[:], dt_[:],
                                             start=False, stop=True)
                            ot = pout2.tile([16, 320], f32, tag="cout", name="coutc")
                            nc.scalar.activation(ot[:], ps[:, :], ACTF.Relu)
                            weng = nc.sync if g % 2 == 0 else nc.gpsimd
                            weng.dma_start(
                                out=dap(outp, (16 * g) * W + xo, [(W, 16), (1, 320)]),
                                in_=ot[:])

                    convc_half(0, 0, 321)
                    convc_half(320, 321, 319)

    # Engine-sem update thinning: Tile increments each engine's sem on every
    # op, but only values that some wait references matter. Dropping the rest
    # (and renumbering waits to the kept-update count at the same producer op)
    # is semantically exact and shrinks the sem-update backlog.
    ENG_SEMS = ("DVE_44", "PE_44", "Activation_44", "Pool_44", "SP_44")
    insts_all = []
    for fn in nc.m.functions:
        for bb in fn.blocks:
            insts_all.extend(bb.instructions)
    waited = {sm: set() for sm in ENG_SEMS}
    for inst in insts_all:
        si = inst.sync_info
        if si and si.on_wait:
            for wt_ in si.on_wait:
                if wt_.ant_name in waited:
                    waited[wt_.ant_name].add(wt_.wait_value)
    cum = {sm: 0 for sm in ENG_SEMS}
    newcum = {sm: 0 for sm in ENG_SEMS}
    remap = {sm: {} for sm in ENG_SEMS}
    for inst in insts_all:
        si = inst.sync_info
        if not si:
            continue
        ups = si.on_update
        if ups:
            keep = []
            for u in ups:
                sm = u.ant_name
                if sm in cum:
                    cum[sm] += 1
                    if cum[sm] in waited[sm]:
                        newcum[sm] += 1
                        remap[sm][cum[sm]] = newcum[sm]
                        keep.append(u)
                    # else: drop this update
                else:
                    keep.append(u)
            if len(keep) != len(ups):
                si.on_update = keep
    for inst in insts_all:
        si = inst.sync_info
        if si and si.on_wait:
            ws = list(si.on_wait)
            ch = False
            for i_, wt_ in enumerate(ws):
                if wt_.ant_name in remap and wt_.wait_value in remap[wt_.ant_name]:
                    nv = remap[wt_.ant_name][wt_.wait_value]
                    if nv != wt_.wait_value:
                        wt_.wait_value = nv
                        ch = True
            if ch:
                si.on_wait = ws

    # walrus on this stack accepts at most one sync wait per instruction:
    # spill excess waits onto same-engine NOPs injected just before.
    nwn = [0]
    for fn in nc.m.functions:
        for bb in fn.blocks:
            il = bb.instructions
            i = 0
            while i < len(il):
                inst = il[i]
                si = inst.sync_info
                if si is not None and si.on_wait and len(si.on_wait) > 1:
                    waits = list(si.on_wait)
                    si.on_wait = waits[-1:]
                    for j in range(len(waits) - 1):
                        nwn[0] += 1
                        nop = mybir.InstNoOp(name=f"WS-{nwn[0]}", ins=[], outs=[])
                        nop.engine = inst.engine
                        nop.sync_info = mybir.SyncInfo(on_wait=[waits[j]], on_update=[])
                        nc.register_instruction(nop, overwrite=True)
                        il.insert(i, nop)
                        i += 1
                i += 1

    return nc


def _prep_inputs(inputs):
    w1, b1 = _fold_bn(inputs['w1'], inputs['bn1_g'], inputs['bn1_b'], inputs['bn1_m'], inputs['bn1_v'])
    w2, b2 = _fold_bn(inputs['w2'], inputs['bn2_g'], inputs['bn2_b'], inputs['bn2_m'], inputs['bn2_v'])
    w3, b3 = _fold_bn(inputs['w3'], inputs['bn3_g'], inputs['bn3_b'], inputs['bn3_m'], inputs['bn3_v'])

    w1k = _lhsT(w1, 8, 10, 3, 1).astype(BF)                       # [30, 384]
    w2k = _lhsT(w2, 6, 8, 16, 1).astype(BF)                       # [128, 288]
    w3k = _lhsT(w3, 6, 8, 16, 1).astype(BF)
    w4k = _lhsT(inputs['w4'].astype(np.float32), 5, 7, 16, 1).astype(BF)   # [112, 360]
    wdk = _lhsT(inputs['wd'].astype(np.float32), 16, 18, 1, 1).astype(BF)  # [18, 384]
    wck = _lhsT(inputs['wc'].astype(np.float32), 16, 18, 4, 2).astype(BF)  # [72, 96]

    b1r = np.tile(b1, 8).reshape(128, 1).astype(np.float32)
    b2r = np.tile(b2, 6).reshape(96, 1).astype(np.float32)
    b3r = np.tile(b3, 6).reshape(96, 1).astype(np.float32)

    maps = []
    for b in range(8):
        maps.append({
            "img": _pad_rowi(inputs['leftImage'][b]),
            "dpad": _pad_rowi(inputs['disp'][b]),
            "dispb": inputs['disp'][b, 0].astype(BF),
            "w1k": w1k, "w2k": w2k, "w3k": w3k, "w4k": w4k, "wdk": wdk, "wck": wck,
            "b1v": b1r, "b2v": b2r, "b3v": b3r, "idv": np.eye(16, dtype=BF),
        })
    return maps


def kernel(**inputs):
    from concourse.bass_utils import run_bass_kernel_spmd

    if "nc" not in _CACHE:
        _CACHE["nc"] = _build()
    nc = _CACHE["nc"]
    maps = _prep_inputs(inputs)
    res = run_bass_kernel_spmd(nc, maps, core_ids=list(range(8)))
    out = np.stack([res.results[i]["out"] for i in range(8)])[:, None].astype(np.float32)
    return out


# revision 68
# speedup vs baseline: 3.4450x; 1.0596x over previous
"""Trainium2 Bass kernel for nn_AnyNetRefinement (disparity refinement with SPN scan).

Data-parallel over batch: core b processes image b end-to-end (no collectives).

v2: all DRAM round-trips use row-interleaved layouts ([row, channel, col]) with
matmul M-order (row, channel), so every conv load/store DMA collapses to a
2-dim AP [(WP, M), (1, cols)] (single-descriptor-chain cost). Epilogues on
gpsimd, writes on scalar queue, loads on sync queue. Gate normalization is
chunked per output row and balanced across Act/Pool/DVE. The SPN scan is the
folded [128=(c,hblock), 26] bf16 state machine (3-tap multiply + 4-slot
reduce + halo shuffles). Prop is exported to a channel-planar DRAM buffer in
24 merged per-block-row DMAs and convc consumes it directly.
"""

import numpy as np
import ml_dtypes

BF = ml_dtypes.bfloat16

H, W = 384, 640
HP, WP = 387, 642        # padded rows (+1 top, +2 bottom), cols (+1 left, +1 right)
NX = 320                 # half free width (psum-bank safe: 512+128 split)

_CACHE = {}


# ---------------------------------------------------------------- host helpers
def _fold_bn(wt, g, b, m, v):
    s = g / np.sqrt(v + 1e-5)
    return (wt * s.reshape(-1, 1, 1, 1)).astype(np.float32), (b - m * s).astype(np.float32)


def _lhsT(wt, r_out, r_in, cin_g, npass):
    """lhsT [K=(cin_g,yi), npass, 3, M=(yo,cout)] -- M row-major."""
    cout, cin = wt.shape[0], wt.shape[1]
    K = cin_g * r_in
    M = cout * r_out
    out = np.zeros((K, npass, 3, M), np.float32)
    for p in range(npass):
        for cg in range(cin_g):
            c = p * cin_g + cg
            if c >= cin:
                continue
            for dx in range(3):
                for yi in range(r_in):
                    k = cg * r_in + yi
                    for co in range(cout):
                        for yo in range(r_out):
                            dy = yi - yo
                            if 0 <= dy <= 2:
                                out[k, p, dx, yo * cout + co] = wt[co, c, dy, dx]
    return out.reshape(K, npass * 3 * M)


def _lhsT_dx(wt, r_out, r_in):
    """dx folded into K: lhsT [K=(dx,cin,yi), M=(yo,cout)] (single matmul/half)."""
    cout, cin = wt.shape[0], wt.shape[1]
    Kb = cin * r_in
    M = cout * r_out
    out = np.zeros((3 * Kb, M), np.float32)
    for dx in range(3):
        for cg in range(cin):
            for yi in range(r_in):
                k = dx * Kb + cg * r_in + yi
                for co in range(cout):
                    for yo in range(r_out):
                        dy = yi - yo
                        if 0 <= dy <= 2:
                            out[k, yo * cout + co] = wt[co, cg, dy, dx]
    return out


def _pad_rowi(x):
    """[C, H, W] -> row-interleaved padded [HP, C, WP] bf16."""
    c = x.shape[0]
    out = np.zeros((HP, c, WP), BF)
    out[1:1 + H, :, 1:1 + W] = np.moveaxis(x, 0, 1).astype(BF)
    return out


# ---------------------------------------------------------------- bass builder
def _build():
    import concourse.bass as bass
    import concourse.mybir as mybir
    from concourse import tile
    from concourse.vector_clock import ScopedClock

    f32 = mybir.dt.float32
    bf16 = mybir.dt.bfloat16
    ALU = mybir.AluOpType
    ACTF = mybir.ActivationFunctionType
    AX = mybir.AxisListType

    class TC(tile.TileContext):
        # this walrus build accepts only one sync-wait per Drain; split the
        # end-of-kernel waits across several drains.
        def _drain_and_barrier(self, tick_clock, wait_clock):
            nc = self.nc
            drain_inst = nc.sync.drain()
            wait_clock.add_sem_waits(drain_inst.ins, ScopedClock({None: tick_clock.global_clock}))
            waits = list(drain_inst.ins.sync_info.on_wait)
            if len(waits) > 1:
                drain_inst.ins.sync_info.on_wait = waits[:1]
                for i in range(1, len(waits)):
                    d2 = nc.sync.drain()
                    if d2.ins.sync_info is None:
                        d2.ins.sync_info = mybir.SyncInfo(on_wait=[waits[i]], on_update=[])
                    else:
                        d2.ins.sync_info.on_wait = [waits[i]]
            nc.all_engine_barrier()
            popped = nc._tile_sem_poison_stack.pop()
            assert popped is self._sem_poison
            nc.clear_and_free_semaphores(list(self.sems.allocated().values()))
            nc.all_engine_barrier()

    def dap(t, offset, dims):
        base = t if isinstance(t, bass.AP) else t[:]
        return bass.AP(base.tensor, base.offset + offset, [list(d) for d in dims])

    def sap(tile_ap, nparts, offset, dims, pstride=1):
        pstep = tile_ap.ap[0][0]
        return bass.AP(tile_ap.tensor, tile_ap.offset + offset,
                       [[pstep * pstride, nparts]] + [list(d) for d in dims])

    nc = bass.Bass("TRN2", num_swdge_queues=4)

    img = nc.declare_dram_parameter("img", [HP, 3, WP], bf16, isOutput=False)
    dpad = nc.declare_dram_parameter("dpad", [HP, 1, WP], bf16, isOutput=False)
    dispb = nc.declare_dram_parameter("dispb", [H, W], bf16, isOutput=False)
    w1k = nc.declare_dram_parameter("w1k", [30, 3 * 128], bf16, isOutput=False)
    w2k = nc.declare_dram_parameter("w2k", [128, 3 * 96], bf16, isOutput=False)
    w3k = nc.declare_dram_parameter("w3k", [128, 3 * 96], bf16, isOutput=False)
    w4k = nc.declare_dram_parameter("w4k", [112, 3 * 120], bf16, isOutput=False)
    wdk = nc.declare_dram_parameter("wdk", [54, 128], bf16, isOutput=False)
    wck = nc.declare_dram_parameter("wck", [72, 2 * 3 * 16], bf16, isOutput=False)
    b1v = nc.declare_dram_parameter("b1v", [128, 1], f32, isOutput=False)
    b2v = nc.declare_dram_parameter("b2v", [96, 1], f32, isOutput=False)
    b3v = nc.declare_dram_parameter("b3v", [96, 1], f32, isOutput=False)
    idv = nc.declare_dram_parameter("idv", [16, 16], bf16, isOutput=False)
    outp = nc.declare_dram_parameter("out", [H, W], f32, isOutput=True)

    with TC(nc) as tc:
        with (tc.tile_pool(name="dram", bufs=1, space="DRAM") as dram,
              tc.tile_pool(name="wts", bufs=1) as pw):
            act1 = dram.tile([HP, 16, WP], bf16, tag="act1")
            act2 = dram.tile([HP, 16, WP], bf16, tag="act2")
            act3 = dram.tile([HP, 16, WP], bf16, tag="act3")
            Gt = dram.tile([385, 24, W], bf16, tag="G")
            dfeat = dram.tile([H, 8, W], bf16, tag="dfeat")
            prop = dram.tile([8, H, WP], bf16, tag="prop")

            # ---------------- weights/biases
            wtl = {}
            for nm, prm, kk, nm3 in (("w1", w1k, 30, 3 * 128), ("w2", w2k, 128, 3 * 96),
                                     ("w3", w3k, 128, 3 * 96), ("w4", w4k, 112, 3 * 120),
                                     ("wd", wdk, 54, 128), ("wc", wck, 72, 6 * 16)):
                t = pw.tile([kk, nm3], bf16, tag=f"{nm}t", name=f"{nm}t")
                nc.sync.dma_start(out=t[:], in_=prm[:])
                wtl[nm] = t
            b1t = pw.tile([128, 1], f32, tag="b1t")
            nc.sync.dma_start(out=b1t[:], in_=b1v[:])
            b2t = pw.tile([96, 1], f32, tag="b2t")
            nc.sync.dma_start(out=b2t[:], in_=b2v[:])
            b3t = pw.tile([96, 1], f32, tag="b3t")
            nc.sync.dma_start(out=b3t[:], in_=b3v[:])
            idt = pw.tile([16, 16], bf16, tag="idt")
            nc.sync.dma_start(out=idt[:], in_=idv[:])
            epst = pw.tile([128, 1], f32, tag="epst")
            nc.gpsimd.memset(epst[:], 1e-8)

            # ---------------- zero borders of padded DRAM buffers
            zt = pw.tile([128, WP], bf16, tag="zt")
            nc.vector.memset(zt[:], 0.0)
            ztv = zt[:]
            for buf in (act1, act2, act3):
                # row 0 (16ch) and rows 385,386 (32 rows*ch), full padded width
                nc.gpsimd.dma_start(out=dap(buf, 0, [(WP, 16), (1, WP)]),
                                    in_=sap(ztv, 16, 0, [(1, WP)]))
                nc.gpsimd.dma_start(out=dap(buf, 385 * 16 * WP, [(WP, 32), (1, WP)]),
                                    in_=sap(ztv, 32, 0, [(1, WP)]))
                # cols 0 and WP-1, rows 1..384 all ch (merged (row,ch) stride WP)
                for col in (0, WP - 1):
                    nc.gpsimd.dma_start(
                        out=dap(buf, 16 * WP + col, [(WP, 16 * H), (1, 1)]),
                        in_=sap(ztv, 128, 0, [(0, 48), (1, 1)]))
            # prop: cols 0/641, all rows*channels (merged stride WP)
            for col in (0, WP - 1):
                nc.gpsimd.dma_start(
                    out=dap(prop, col, [(WP, 8 * H), (1, 1)]),
                    in_=sap(ztv, 128, 0, [(0, 24), (1, 1)]))

            # ---------------- generic conv (row-group pipelined)
            def conv(prhs, ppsum, pout, wt, K, M, npass, S, load, dst_ap, epi,
                     odt, st, gr=None):
                wv = wt[:]
                for g in (gr if gr is not None else range(S)):
                    rhss = []
                    for p_ in range(npass):
                        rt = prhs.tile([K, WP], bf16, tag="rhs", name="rhs")
                        load(rt, p_, g)
                        rhss.append(rt)
                    ps = ppsum.tile([128, 2 * NX], f32, tag="ps", name="ps")
                    for x0, nxw in ((0, 512), (512, 128)):
                        for p_ in range(npass):
                            for dx in range(3):
                                nc.tensor.matmul(
                                    ps[:M, x0:x0 + nxw],
                                    sap(wv, K, (p_ * 3 + dx) * M, [(1, M)]),
                                    rhss[p_][:, x0 + dx:x0 + dx + nxw],
                                    start=(p_ == 0 and dx == 0),
                                    stop=(p_ == npass - 1 and dx == 2))
                    ot = pout.tile([M, 2 * NX], odt, tag="cout", name="cout")
                    epi(ps, ot, g)
                    weng = st if g % 2 == 0 else nc.gpsimd
                    weng.dma_start(out=dst_ap(g), in_=ot[:])

            with (tc.tile_pool(name="rhs", bufs=12) as prhs,
                  tc.tile_pool(name="cout", bufs=12) as pout,
                  tc.tile_pool(name="psum", bufs=4, space="PSUM") as ppsum):
                # NOTE: gpsimd cannot touch PSUM (BIR verifier) -- epilogues
                # that read PSUM live on DVE (tensor_scalar) or Act (activation
                # relu-with-bias), split 2:1 to keep both under PE.
                def bias_relu(bt, M):
                    def epi(ps, ot, g):
                        if g % 3 == 2:
                            nc.scalar.activation(ot[:], ps[:M, :], ACTF.Relu,
                                                 bias=bt[:M, :])
                        else:
                            nc.vector.tensor_scalar(ot[:], ps[:M, :], bt[:M, :], 0.0,
                                                    ALU.add, ALU.max)
                    return epi

                def copy_epi(M):
                    def epi(ps, ot, g):
                        if g % 3 == 2:
                            nc.scalar.activation(ot[:], ps[:M, :], ACTF.Copy)
                        else:
                            nc.vector.tensor_copy(ot[:], ps[:M, :])
                    return epi

                def simple_load(src, cg_n, rin, r, cpitch):
                    def load(rt, p_, g):
                        nc.sync.dma_start(
                            out=rt[:],
                            in_=dap(src, (r * g) * cpitch * WP,
                                    [(WP, cg_n), (cpitch * WP, rin), (1, WP)])
                            if cg_n > 1 else
                            dap(src, (r * g) * WP, [(WP, rin), (1, WP)]))
                    return load

                # dx folded into K (3 shifted rhs copies): one matmul per half
                def convf(wt, Kb, M, S, r, rin, cin, src, cpitch, dst_ap, epi, odt,
                          gr=None):
                    wv = wt[:]
                    engs = (nc.sync, nc.scalar, nc.gpsimd)
                    for g in (gr if gr is not None else range(S)):
                        rt = prhs.tile([3 * Kb, 640], bf16, tag="rhs", name="rhsf")
                        rv = rt[:]
                        pstep = rv.ap[0][0]
                        for dx in range(3):
                            engs[dx].dma_start(
                                out=bass.AP(rv.tensor, rv.offset + (dx * Kb) * pstep,
                                            [[pstep, Kb], [1, 640]]),
                                in_=dap(src, (r * g) * cpitch * WP + dx,
                                        [(WP, cin), (cpitch * WP, rin), (1, 640)])
                                if cin > 1 else
                                dap(src, (r * g) * WP + dx, [(WP, rin), (1, 640)]))
                        ps = ppsum.tile([128, 2 * NX], f32, tag="ps", name="ps")
                        for x0, nxw in ((0, 512), (512, 128)):
                            nc.tensor.matmul(ps[:M, x0:x0 + nxw],
                                             sap(wv, 3 * Kb, 0, [(1, M)]),
                                             rt[:, x0:x0 + nxw],
                                             start=True, stop=True)
                        ot = pout.tile([M, 2 * NX], odt, tag="cout", name="cout")
                        epi(ps, ot, g)
                        weng = nc.scalar if g % 2 == 0 else nc.gpsimd
                        weng.dma_start(out=dst_ap(g), in_=ot[:])

                # conv1: img -> act1  (K=30=(3cg,10yi), M=128=(8yo,16c))
                conv(prhs, ppsum, pout, wtl["w1"], 30, 128, 1, 48,
                     simple_load(img, 3, 10, 8, 3),
                     lambda g: dap(act1, (1 + 8 * g) * 16 * WP + 1,
                                   [(WP, 128), (1, 640)]),
                     bias_relu(b1t, 128), bf16, nc.scalar)
                # conv2: act1 -> act2  (K=128=(16cg,8yi), M=96=(6yo,16c)), with
                # convd slices interleaved so convd's DMAs hide in conv2's slack
                def conv2_chunk(gr):
                    conv(prhs, ppsum, pout, wtl["w2"], 128, 96, 1, 64,
                         simple_load(act1, 16, 8, 6, 16),
                         lambda g: dap(act2, (1 + 6 * g) * 16 * WP + 1,
                                       [(WP, 96), (1, 640)]),
                         bias_relu(b2t, 96), bf16, nc.scalar, gr=gr)

                def convd_chunk(gr):
                    convf(wtl["wd"], 18, 128, 24, 16, 18, 1, dpad, 1,
                          lambda g: dap(dfeat, (16 * g) * 8 * W, [(W, 128), (1, 640)]),
                          copy_epi(128), bf16, gr=gr)

                for ck in range(4):
                    conv2_chunk(range(16 * ck, 16 * ck + 16))
                    convd_chunk(range(6 * ck, 6 * ck + 6))
                # conv3: act2 -> act3
                conv(prhs, ppsum, pout, wtl["w3"], 128, 96, 1, 64,
                     simple_load(act2, 16, 8, 6, 16),
                     lambda g: dap(act3, (1 + 6 * g) * 16 * WP + 1,
                                   [(WP, 96), (1, 640)]),
                     bias_relu(b3t, 96), bf16, nc.scalar)
                # conv4: act3 -> Gt (K=112=(16cg,7yi), M=120=(5yo,24c))
                conv(prhs, ppsum, pout, wtl["w4"], 112, 120, 1, 77,
                     simple_load(act3, 16, 7, 5, 16),
                     lambda g: dap(Gt, (5 * g) * 24 * W, [(W, 120), (1, 640)]),
                     copy_epi(120), bf16, nc.scalar)

            # ---------------- scan-resident gate/w0 tiles + normalization
            with tc.tile_pool(name="scanbig", bufs=1) as pbig:
                # gw tap-fastest [24 rows, W cols, 3 taps]; pf column-major
                # [WP cols, 26 rows] -- every scan AP is then innermost-
                # contiguous, which unlocks the DVE 2x bf16 mode for the
                # per-step 3-tap multiply.
                gw = pbig.tile([128, 24, W, 3], bf16, tag="gw")
                gwv = gw[:]
                w0w = pbig.tile([128, 24, W], bf16, tag="w0w")
                w0v = w0w[:]
                TB = 16  # w0-staging chunk
                pf = pbig.tile([128, WP, 26], bf16, tag="pf")
                pfv = pf[:]
                # only col 0 (initial state + first-step halos) must be zero;
                # every other cell is written before it is read.
                nc.vector.memset(sap(pfv, 128, 0, [(1, 26)]), 0.0)
                prw = pbig.tile([128, 6, TB, 24, 4], bf16, tag="prw")
                prv = prw[:]

                # normalization: per output row r (24 rows per h-block), partitions
                # are (c:8, hb:16); Gt row-interleaved [385,24,W] gives 3-dim APs.
                with (tc.tile_pool(name="norm3", bufs=6) as pn3,
                      tc.tile_pool(name="norm1", bufs=2) as pn1,
                      tc.tile_pool(name="normf", bufs=3) as pnf):
                    for r in range(24):
                        gl = []
                        for tap in range(3):
                            g2 = pn3.tile([128, W], bf16, tag="gld", name="gld")
                            nc.sync.dma_start(
                                out=g2[:],
                                in_=dap(Gt, r * 24 * W + tap * 8 * W,
                                        [(W, 8), (24 * 24 * W, 16), (1, W)]))
                            gl.append(g2)
                        df = pn1.tile([128, W], bf16, tag="dfl", name="dfl")
                        nc.sync.dma_start(
                            out=df[:],
                            in_=dap(dfeat, r * 8 * W, [(W, 8), (24 * 8 * W, 16), (1, W)]))
                        ab = []
                        for tap in range(3):
                            a = pn3.tile([128, W], bf16, tag="gabs", name="gabs")
                            # |G2 + 1e-8| != |G2| only within 1e-8; keeps the
                            # div-by-zero guard without a separate epsilon op
                            # (1e-8 is not bf16-representable, so G2+1e-8 can
                            # never cancel to exactly 0).
                            nc.scalar.activation(a[:], gl[tap][:], ACTF.Abs,
                                                 bias=epst[:] if tap == 2 else 0.0)
                            ab.append(a)
                        s12 = pn1.tile([128, W], bf16, tag="s12")
                        nc.vector.tensor_tensor(out=s12[:], in0=ab[0][:], in1=ab[1][:],
                                                op=ALU.add)
                        sf = pnf.tile([128, W], f32, tag="sf")
                        nc.gpsimd.tensor_tensor(out=sf[:], in0=ab[2][:], in1=s12[:],
                                                op=ALU.add)
                        nc.vector.reciprocal(sf[:], sf[:])
                        for tap, eng in zip(range(3), (nc.gpsimd, nc.gpsimd, nc.vector)):
                            o = sap(gwv, 128, r * (W * 3) + tap, [(3, W)])
                            eng.tensor_tensor(out=o, in0=gl[tap][:], in1=sf[:],
                                              op=ALU.mult)
                        # t2 must come from the ROUNDED bf16 gates so the scan
                        # coefficients sum to exactly 1 (else drift accumulates
                        # over the 640-step scan).
                        gs0 = sap(gwv, 128, r * (W * 3) + 0, [(3, W)])
                        gs1 = sap(gwv, 128, r * (W * 3) + 1, [(3, W)])
                        gs2 = sap(gwv, 128, r * (W * 3) + 2, [(3, W)])
                        a12 = pn1.tile([128, W], bf16, tag="a12")
                        nc.gpsimd.tensor_tensor(out=a12[:], in0=gs0, in1=gs1,
                                                op=ALU.add)
                        nc.vector.tensor_tensor(out=a12[:], in0=a12[:], in1=gs2,
                                                op=ALU.add)
                        nc.vector.tensor_scalar(a12[:], a12[:], -1.0, 1.0,
                                                ALU.mult, ALU.add)
                        nc.gpsimd.tensor_tensor(out=sap(w0v, 128, r * W, [(1, W)]),
                                                in0=a12[:], in1=df[:], op=ALU.mult)
                    # zero edge gates: tap0 (up) row 0 of hb=0; tap2 (dn) row 23 of hb=15
                    pstep = gwv.ap[0][0]
                    nc.sync.dma_start(
                        out=bass.AP(gwv.tensor, gwv.offset,
                                    [[16 * pstep, 8], [3, W], [1, 1]]),
                        in_=sap(ztv, 8, 0, [(1, W), (0, 1)]))
                    for c_ in range(8):
                        nc.sync.dma_start(
                            out=bass.AP(gwv.tensor,
                                        gwv.offset + (16 * c_ + 15) * pstep
                                        + 23 * (W * 3) + 2,
                                        [[pstep, 1], [3, W], [1, 1]]),
                            in_=sap(ztv, 1, 0, [(1, W), (0, 1)]))

                # ---------------- SPN scan (pf column-major: elem (x, r) at x*26+r)
                mask_up = [(i - 1) % 32 for i in range(32)]
                mask_dn = [(i + 1) % 32 for i in range(32)]
                for t in range(W):
                    j = t % TB
                    bi = (t // TB) % 6
                    if j == 0:
                        nc.scalar.copy(
                            sap(prv, 128, bi * (TB * 96) + 3, [(4, 24), (96, TB)]),
                            sap(w0v, 128, t, [(W, 24), (1, TB)]))
                    base = bi * (TB * 96) + j * 96
                    taps = sap(pfv, 128, t * 26, [(1, 24), (1, 3)])
                    g_t = sap(gwv, 128, t * 3, [(3 * W, 24), (1, 3)])
                    nc.vector.tensor_tensor(out=sap(prv, 128, base, [(4, 24), (1, 3)]),
                                            in0=g_t, in1=taps, op=ALU.mult)
                    with nc.allow_low_precision(reason="bf16 scan state, validated"):
                        nc.vector.tensor_reduce(out=sap(pfv, 128, (1 + t) * 26 + 1, [(1, 24)]),
                                                in_=sap(prv, 128, base, [(4, 24), (1, 4)]),
                                                axis=AX.X, op=ALU.add)
                    nc.vector.stream_shuffle(out=sap(pfv, 128, (1 + t) * 26, [(1, 1)]),
                                             in_=sap(pfv, 128, (1 + t) * 26 + 24, [(1, 1)]),
                                             mask=mask_up)
                    nc.vector.stream_shuffle(out=sap(pfv, 128, (1 + t) * 26 + 25, [(1, 1)]),
                                             in_=sap(pfv, 128, (1 + t) * 26 + 1, [(1, 1)]),
                                             mask=mask_dn)
                # ---------------- prop export + convc, split into column halves:
                # half A (out cols 0..319) depends only on scan steps <= 321,
                # so its exports/loads/matmuls/epilogue all run during the
                # scan's second half on the otherwise-idle engines.
                with (tc.tile_pool(name="rhsc", bufs=8) as prhs2,
                      tc.tile_pool(name="coutc", bufs=8) as pout2,
                      tc.tile_pool(name="psumc", bufs=4, space="PSUM") as ppsum2):
                    wcv = wtl["wc"][:]

                    def convc_part(xo, xw, ex0, exn, in_scan):
                        # export data cols ex0..ex0+exn-1 (prop col = x+1).
                        # While the scan is live, Act runs ONLY the relus (so
                        # w0-staging copies on Act are never starved); exports
                        # go to SP, dtile/writes to Pool.
                        nw = xw + 2
                        with nc.allow_non_contiguous_dma(reason="pf col-major"):
                            for r in range(24):
                                eng = nc.sync if (in_scan or r % 2 == 0) else nc.scalar
                                eng.dma_start(
                                    out=dap(prop, r * WP + 1 + ex0,
                                            [(24 * WP, 128), (1, exn)]),
                                    in_=sap(pfv, 128, (1 + ex0) * 26 + 1 + r,
                                            [(26, exn), (1, 1)]))
                        c0 = xo  # first prop col in rhs tiles
                        for g in range(24):
                            rhss = []
                            for p_ in range(2):
                                ld = nc.sync if (in_scan or p_ == 0) else nc.scalar
                                rt = prhs2.tile([72, nw], bf16, tag="rhs", name="rhsc")
                                rv = rt[:]
                                pstep = rv.ap[0][0]
                                if g == 0 or g == 23:
                                    zrow = 0 if g == 0 else 17
                                    drow0, prow0 = (1, 0) if g == 0 else (0, 367)
                                    for cg in range(4):
                                        nc.gpsimd.dma_start(
                                            out=bass.AP(rv.tensor,
                                                        rv.offset + (cg * 18 + zrow) * pstep,
                                                        [[pstep, 1], [1, nw]]),
                                            in_=sap(ztv, 1, 0, [(1, nw)]))
                                        ld.dma_start(
                                            out=bass.AP(rv.tensor,
                                                        rv.offset + (cg * 18 + drow0) * pstep,
                                                        [[pstep, 17], [1, nw]]),
                                            in_=dap(prop,
                                                    (p_ * 4 + cg) * H * WP
                                                    + prow0 * WP + c0,
                                                    [(WP, 17), (1, nw)]))
                                else:
                                    ld.dma_start(
                                        out=rv,
                                        in_=dap(prop,
                                                p_ * 4 * H * WP + (16 * g - 1) * WP + c0,
                                                [(H * WP, 4), (WP, 18), (1, nw)]))
                                rhss.append(rt)
                            # disp residual enters PSUM via an identity matmul,
                            # so the epilogue is a single Act relu.
                            dt_ = pout2.tile([16, xw], bf16, tag="dtile", name="dtile")
                            nc.gpsimd.dma_start(
                                out=dt_[:],
                                in_=dap(dispb, (16 * g) * W + xo, [(W, 16), (1, xw)]))
                            ps = ppsum2.tile([16, 512], f32, tag="psc", name="psc")
                            for p_ in range(2):
                                for dx in range(3):
                                    nc.tensor.matmul(
                                        ps[:, :xw],
                                        sap(wcv, 72, (p_ * 3 + dx) * 16, [(1, 16)]),
                                        rhss[p_][:, dx:dx + xw],
                                        start=(p_ == 0 and dx == 0),
                                        stop=False)
                            nc.tensor.matmul(ps[:, :xw], idt[:], dt_[:],
                                             start=False, stop=True)
                            ot = pout2.tile([16, xw], f32, tag="cout", name="coutc")
                            if not in_scan and g % 2 == 0:
                                nc.vector.tensor_scalar(ot[:], ps[:, :xw], 0.0, None,
                                                        ALU.max)
                            else:
                                nc.scalar.activation(ot[:], ps[:, :xw], ACTF.Relu)
                            if in_scan:
                                weng = nc.gpsimd
                            else:
                                weng = nc.sync if g % 2 == 0 else nc.gpsimd
                            weng.dma_start(
                                out=dap(outp, (16 * g) * W + xo, [(W, 16), (1, xw)]),
                                in_=ot[:])

                    convc_part(0, 214, 0, 215, True)
                    convc_part(214, 214, 215, 214, True)
                    convc_part(428, 212, 429, 211, False)

    # Engine-sem update thinning: Tile increments each engine's sem on every
    # op, but only values that some wait references matter. Dropping the rest
    # (and renumbering waits to the kept-update count at the same producer op)
    # is semantically exact and shrinks the sem-update backlog.
    ENG_SEMS = ("DVE_44", "PE_44", "Activation_44", "Pool_44", "SP_44")
    insts_all = []
    for fn in nc.m.functions:
        for bb in fn.blocks:
            insts_all.extend(bb.instructions)
    waited = {sm: set() for sm in ENG_SEMS}
    for inst in insts_all:
        si = inst.sync_info
        if si and si.on_wait:
            for wt_ in si.on_wait:
                if wt_.ant_name in waited:
                    waited[wt_.ant_name].add(wt_.wait_value)
    cum = {sm: 0 for sm in ENG_SEMS}
    newcum = {sm: 0 for sm in ENG_SEMS}
    remap = {sm: {} for sm in ENG_SEMS}
    for inst in insts_all:
        si = inst.sync_info
        if not si:
            continue
        ups = si.on_update
        if ups:
            keep = []
            for u in ups:
                sm = u.ant_name
                if sm in cum:
                    cum[sm] += 1
                    if cum[sm] in waited[sm]:
                        newcum[sm] += 1
                        remap[sm][cum[sm]] = newcum[sm]
                        keep.append(u)
                    # else: drop this update
                else:
                    keep.append(u)
            if len(keep) != len(ups):
                si.on_update = keep
    for inst in insts_all:
        si = inst.sync_info
        if si and si.on_wait:
            ws = list(si.on_wait)
            ch = False
            for i_, wt_ in enumerate(ws):
                if wt_.ant_name in remap and wt_.wait_value in remap[wt_.ant_name]:
                    nv = remap[wt_.ant_name][wt_.wait_value]
                    if nv != wt_.wait_value:
                        wt_.wait_value = nv
                        ch = True
            if ch:
                si.on_wait = ws

    # walrus on this stack accepts at most one sync wait per instruction:
    # spill excess waits onto same-engine NOPs injected just before.
    nwn = [0]
    for fn in nc.m.functions:
        for bb in fn.blocks:
            il = bb.instructions
            i = 0
            while i < len(il):
                inst = il[i]
                si = inst.sync_info
                if si is not None and si.on_wait and len(si.on_wait) > 1:
                    waits = list(si.on_wait)
                    si.on_wait = waits[-1:]
                    for j in range(len(waits) - 1):
                        nwn[0] += 1
                        nop = mybir.InstNoOp(name=f"WS-{nwn[0]}", ins=[], outs=[])
                        nop.engine = inst.engine
                        nop.sync_info = mybir.SyncInfo(on_wait=[waits[j]], on_update=[])
                        nc.register_instruction(nop, overwrite=True)
                        il.insert(i, nop)
                        i += 1
                i += 1

    return nc


def _prep_inputs(inputs):
    w1, b1 = _fold_bn(inputs['w1'], inputs['bn1_g'], inputs['bn1_b'], inputs['bn1_m'], inputs['bn1_v'])
    w2, b2 = _fold_bn(inputs['w2'], inputs['bn2_g'], inputs['bn2_b'], inputs['bn2_m'], inputs['bn2_v'])
    w3, b3 = _fold_bn(inputs['w3'], inputs['bn3_g'], inputs['bn3_b'], inputs['bn3_m'], inputs['bn3_v'])

    w1k = _lhsT(w1, 8, 10, 3, 1).astype(BF)                       # [30, 384]
    w2k = _lhsT(w2, 6, 8, 16, 1).astype(BF)                       # [128, 288]
    w3k = _lhsT(w3, 6, 8, 16, 1).astype(BF)
    w4k = _lhsT(inputs['w4'].astype(np.float32), 5, 7, 16, 1).astype(BF)   # [112, 360]
    wdk = _lhsT_dx(inputs['wd'].astype(np.float32), 16, 18).astype(BF)     # [54, 128]
    wck = _lhsT(inputs['wc'].astype(np.float32), 16, 18, 4, 2).astype(BF)  # [72, 96]

    b1r = np.tile(b1, 8).reshape(128, 1).astype(np.float32)
    b2r = np.tile(b2, 6).reshape(96, 1).astype(np.float32)
    b3r = np.tile(b3, 6).reshape(96, 1).astype(np.float32)

    maps = []
    for b in range(8):
        maps.append({
            "img": _pad_rowi(inputs['leftImage'][b]),
            "dpad": _pad_rowi(inputs['disp'][b]),
            "dispb": inputs['disp'][b, 0].astype(BF),
            "w1k": w1k, "w2k": w2k, "w3k": w3k, "w4k": w4k, "wdk": wdk, "wck": wck,
            "b1v": b1r, "b2v": b2r, "b3v": b3r, "idv": np.eye(16, dtype=BF),
        })
    return maps


def kernel(**inputs):
    from concourse.bass_utils import run_bass_kernel_spmd

    if "nc" not in _CACHE:
        _CACHE["nc"] = _build()
    nc = _CACHE["nc"]
    maps = _prep_inputs(inputs)
    res = run_bass_kernel_spmd(nc, maps, core_ids=list(range(8)))
    out = np.stack([res.results[i]["out"] for i in range(8)])[:, None].astype(np.float32)
    return out


# revision 77
# speedup vs baseline: 3.5040x; 1.0171x over previous
"""Trainium2 Bass kernel for nn_AnyNetRefinement (disparity refinement with SPN scan).

Data-parallel over batch: core b processes image b end-to-end (no collectives).

v2: all DRAM round-trips use row-interleaved layouts ([row, channel, col]) with
matmul M-order (row, channel), so every conv load/store DMA collapses to a
2-dim AP [(WP, M), (1, cols)] (single-descriptor-chain cost). Epilogues on
gpsimd, writes on scalar queue, loads on sync queue. Gate normalization is
chunked per output row and balanced across Act/Pool/DVE. The SPN scan is the
folded [128=(c,hblock), 26] bf16 state machine (3-tap multiply + 4-slot
reduce + halo shuffles). Prop is exported to a channel-planar DRAM buffer in
24 merged per-block-row DMAs and convc consumes it directly.
"""

import numpy as np
import ml_dtypes

BF = ml_dtypes.bfloat16

H, W = 384, 640
HP, WP = 387, 642        # padded rows (+1 top, +2 bottom), cols (+1 left, +1 right)
NX = 320                 # half free width (psum-bank safe: 512+128 split)

_CACHE = {}


# ---------------------------------------------------------------- host helpers
def _fold_bn(wt, g, b, m, v):
    s = g / np.sqrt(v + 1e-5)
    return (wt * s.reshape(-1, 1, 1, 1)).astype(np.float32), (b - m * s).astype(np.float32)


def _lhsT(wt, r_out, r_in, cin_g, npass):
    """lhsT [K=(cin_g,yi), npass, 3, M=(yo,cout)] -- M row-major."""
    cout, cin = wt.shape[0], wt.shape[1]
    K = cin_g * r_in
    M = cout * r_out
    out = np.zeros((K, npass, 3, M), np.float32)
    for p in range(npass):
        for cg in range(cin_g):
            c = p * cin_g + cg
            if c >= cin:
                continue
            for dx in range(3):
                for yi in range(r_in):
                    k = cg * r_in + yi
                    for co in range(cout):
                        for yo in range(r_out):
                            dy = yi - yo
                            if 0 <= dy <= 2:
                                out[k, p, dx, yo * cout + co] = wt[co, c, dy, dx]
    return out.reshape(K, npass * 3 * M)


def _lhsT_dx(wt, r_out, r_in):
    """dx folded into K: lhsT [K=(dx,cin,yi), M=(yo,cout)] (single matmul/half)."""
    cout, cin = wt.shape[0], wt.shape[1]
    Kb = cin * r_in
    M = cout * r_out
    out = np.zeros((3 * Kb, M), np.float32)
    for dx in range(3):
        for cg in range(cin):
            for yi in range(r_in):
                k = dx * Kb + cg * r_in + yi
                for co in range(cout):
                    for yo in range(r_out):
                        dy = yi - yo
                        if 0 <= dy <= 2:
                            out[k, yo * cout + co] = wt[co, cg, dy, dx]
    return out


def _pad_rowi(x):
    """[C, H, W] -> row-interleaved padded [HP, C, WP] bf16."""
    c = x.shape[0]
    out = np.zeros((HP, c, WP), BF)
    out[1:1 + H, :, 1:1 + W] = np.moveaxis(x, 0, 1).astype(BF)
    return out


# ---------------------------------------------------------------- bass builder
def _build():
    import concourse.bass as bass
    import concourse.mybir as mybir
    from concourse import tile
    from concourse.vector_clock import ScopedClock

    f32 = mybir.dt.float32
    bf16 = mybir.dt.bfloat16
    ALU = mybir.AluOpType
    ACTF = mybir.ActivationFunctionType
    AX = mybir.AxisListType

    class TC(tile.TileContext):
        # this walrus build accepts only one sync-wait per Drain; split the
        # end-of-kernel waits across several drains.
        def _drain_and_barrier(self, tick_clock, wait_clock):
            nc = self.nc
            drain_inst = nc.sync.drain()
            wait_clock.add_sem_waits(drain_inst.ins, ScopedClock({None: tick_clock.global_clock}))
            waits = list(drain_inst.ins.sync_info.on_wait)
            if len(waits) > 1:
                drain_inst.ins.sync_info.on_wait = waits[:1]
                for i in range(1, len(waits)):
                    d2 = nc.sync.drain()
                    if d2.ins.sync_info is None:
                        d2.ins.sync_info = mybir.SyncInfo(on_wait=[waits[i]], on_update=[])
                    else:
                        d2.ins.sync_info.on_wait = [waits[i]]
            nc.all_engine_barrier()
            popped = nc._tile_sem_poison_stack.pop()
            assert popped is self._sem_poison
            nc.clear_and_free_semaphores(list(self.sems.allocated().values()))
            nc.all_engine_barrier()

    def dap(t, offset, dims):
        base = t if isinstance(t, bass.AP) else t[:]
        return bass.AP(base.tensor, base.offset + offset, [list(d) for d in dims])

    def sap(tile_ap, nparts, offset, dims, pstride=1):
        pstep = tile_ap.ap[0][0]
        return bass.AP(tile_ap.tensor, tile_ap.offset + offset,
                       [[pstep * pstride, nparts]] + [list(d) for d in dims])

    nc = bass.Bass("TRN2", num_swdge_queues=4)

    img = nc.declare_dram_parameter("img", [HP, 3, WP], bf16, isOutput=False)
    dpad = nc.declare_dram_parameter("dpad", [HP, 1, WP], bf16, isOutput=False)
    dispb = nc.declare_dram_parameter("dispb", [H, W], bf16, isOutput=False)
    w1k = nc.declare_dram_parameter("w1k", [30, 3 * 128], bf16, isOutput=False)
    w2k = nc.declare_dram_parameter("w2k", [128, 3 * 96], bf16, isOutput=False)
    w3k = nc.declare_dram_parameter("w3k", [128, 3 * 96], bf16, isOutput=False)
    w4k = nc.declare_dram_parameter("w4k", [112, 3 * 120], bf16, isOutput=False)
    wdk = nc.declare_dram_parameter("wdk", [54, 128], bf16, isOutput=False)
    wck = nc.declare_dram_parameter("wck", [72, 2 * 3 * 16], bf16, isOutput=False)
    b1v = nc.declare_dram_parameter("b1v", [128, 1], f32, isOutput=False)
    b2v = nc.declare_dram_parameter("b2v", [96, 1], f32, isOutput=False)
    b3v = nc.declare_dram_parameter("b3v", [96, 1], f32, isOutput=False)
    idv = nc.declare_dram_parameter("idv", [16, 16], bf16, isOutput=False)
    outp = nc.declare_dram_parameter("out", [H, W], f32, isOutput=True)

    with TC(nc) as tc:
        with (tc.tile_pool(name="dram", bufs=1, space="DRAM") as dram,
              tc.tile_pool(name="wts", bufs=1) as pw):
            act1 = dram.tile([HP, 16, WP], bf16, tag="act1")
            act2 = dram.tile([HP, 16, WP], bf16, tag="act2")
            act3 = dram.tile([HP, 16, WP], bf16, tag="act3")
            Gt = dram.tile([385, 24, W], bf16, tag="G")
            dfeat = dram.tile([H, 8, W], bf16, tag="dfeat")
            prop = dram.tile([8, H, WP], bf16, tag="prop")

            # ---------------- weights/biases
            wtl = {}
            wengs = (nc.sync, nc.scalar, nc.gpsimd)
            for wi, (nm, prm, kk, nm3) in enumerate(
                    (("w1", w1k, 30, 3 * 128), ("w2", w2k, 128, 3 * 96),
                     ("w3", w3k, 128, 3 * 96), ("w4", w4k, 112, 3 * 120),
                     ("wd", wdk, 54, 128), ("wc", wck, 72, 6 * 16))):
                t = pw.tile([kk, nm3], bf16, tag=f"{nm}t", name=f"{nm}t")
                wengs[wi % 3].dma_start(out=t[:], in_=prm[:])
                wtl[nm] = t
            b1t = pw.tile([128, 1], f32, tag="b1t")
            nc.scalar.dma_start(out=b1t[:], in_=b1v[:])
            b2t = pw.tile([96, 1], f32, tag="b2t")
            nc.gpsimd.dma_start(out=b2t[:], in_=b2v[:])
            b3t = pw.tile([96, 1], f32, tag="b3t")
            nc.scalar.dma_start(out=b3t[:], in_=b3v[:])
            idt = pw.tile([16, 16], bf16, tag="idt")
            nc.gpsimd.dma_start(out=idt[:], in_=idv[:])
            epst = pw.tile([128, 1], f32, tag="epst")
            nc.gpsimd.memset(epst[:], 1e-8)

            # ---------------- zero borders of padded DRAM buffers
            zt = pw.tile([128, WP], bf16, tag="zt")
            nc.vector.memset(zt[:], 0.0)
            ztv = zt[:]
            for buf in (act1, act2, act3):
                # row 0 (16ch) and rows 385,386 (32 rows*ch), full padded width
                nc.gpsimd.dma_start(out=dap(buf, 0, [(WP, 16), (1, WP)]),
                                    in_=sap(ztv, 16, 0, [(1, WP)]))
                nc.gpsimd.dma_start(out=dap(buf, 385 * 16 * WP, [(WP, 32), (1, WP)]),
                                    in_=sap(ztv, 32, 0, [(1, WP)]))
                # cols 0 and WP-1, rows 1..384 all ch (merged (row,ch) stride WP)
                for col in (0, WP - 1):
                    nc.gpsimd.dma_start(
                        out=dap(buf, 16 * WP + col, [(WP, 16 * H), (1, 1)]),
                        in_=sap(ztv, 128, 0, [(0, 48), (1, 1)]))
            # prop: cols 0/641, all rows*channels (merged stride WP)
            for col in (0, WP - 1):
                nc.gpsimd.dma_start(
                    out=dap(prop, col, [(WP, 8 * H), (1, 1)]),
                    in_=sap(ztv, 128, 0, [(0, 24), (1, 1)]))

            # ---------------- generic conv (row-group pipelined)
            def conv(prhs, ppsum, pout, wt, K, M, npass, S, load, dst_ap, epi,
                     odt, st, gr=None):
                wv = wt[:]
                for g in (gr if gr is not None else range(S)):
                    rhss = []
                    for p_ in range(npass):
                        rt = prhs.tile([K, WP], bf16, tag="rhs", name="rhs")
                        load(rt, p_, g)
                        rhss.append(rt)
                    ps = ppsum.tile([128, 2 * NX], f32, tag="ps", name="ps")
                    for x0, nxw in ((0, 512), (512, 128)):
                        for p_ in range(npass):
                            for dx in range(3):
                                nc.tensor.matmul(
                                    ps[:M, x0:x0 + nxw],
                                    sap(wv, K, (p_ * 3 + dx) * M, [(1, M)]),
                                    rhss[p_][:, x0 + dx:x0 + dx + nxw],
                                    start=(p_ == 0 and dx == 0),
                                    stop=(p_ == npass - 1 and dx == 2))
                    ot = pout.tile([M, 2 * NX], odt, tag="cout", name="cout")
                    epi(ps, ot, g)
                    weng = st if g % 2 == 0 else nc.gpsimd
                    weng.dma_start(out=dst_ap(g), in_=ot[:])

            with (tc.tile_pool(name="rhs", bufs=12) as prhs,
                  tc.tile_pool(name="cout", bufs=12) as pout,
                  tc.tile_pool(name="psum", bufs=4, space="PSUM") as ppsum):
                # NOTE: gpsimd cannot touch PSUM (BIR verifier) -- epilogues
                # that read PSUM live on DVE (tensor_scalar) or Act (activation
                # relu-with-bias), split 2:1 to keep both under PE.
                def bias_relu(bt, M):
                    def epi(ps, ot, g):
                        if g % 3 == 2:
                            nc.scalar.activation(ot[:], ps[:M, :], ACTF.Relu,
                                                 bias=bt[:M, :])
                        else:
                            nc.vector.tensor_scalar(ot[:], ps[:M, :], bt[:M, :], 0.0,
                                                    ALU.add, ALU.max)
                    return epi

                def copy_epi(M):
                    def epi(ps, ot, g):
                        if g % 3 == 2:
                            nc.scalar.activation(ot[:], ps[:M, :], ACTF.Copy)
                        else:
                            nc.vector.tensor_copy(ot[:], ps[:M, :])
                    return epi

                def simple_load(src, cg_n, rin, r, cpitch):
                    def load(rt, p_, g):
                        nc.sync.dma_start(
                            out=rt[:],
                            in_=dap(src, (r * g) * cpitch * WP,
                                    [(WP, cg_n), (cpitch * WP, rin), (1, WP)])
                            if cg_n > 1 else
                            dap(src, (r * g) * WP, [(WP, rin), (1, WP)]))
                    return load

                # dx folded into K (3 shifted rhs copies): one matmul per half
                def convf(wt, Kb, M, S, r, rin, cin, src, cpitch, dst_ap, epi, odt,
                          gr=None):
                    wv = wt[:]
                    engs = (nc.sync, nc.scalar, nc.gpsimd)
                    for g in (gr if gr is not None else range(S)):
                        rt = prhs.tile([3 * Kb, 640], bf16, tag="rhs", name="rhsf")
                        rv = rt[:]
                        pstep = rv.ap[0][0]
                        for dx in range(3):
                            engs[dx].dma_start(
                                out=bass.AP(rv.tensor, rv.offset + (dx * Kb) * pstep,
                                            [[pstep, Kb], [1, 640]]),
                                in_=dap(src, (r * g) * cpitch * WP + dx,
                                        [(WP, cin), (cpitch * WP, rin), (1, 640)])
                                if cin > 1 else
                                dap(src, (r * g) * WP + dx, [(WP, rin), (1, 640)]))
                        ps = ppsum.tile([128, 2 * NX], f32, tag="ps", name="ps")
                        for x0, nxw in ((0, 512), (512, 128)):
                            nc.tensor.matmul(ps[:M, x0:x0 + nxw],
                                             sap(wv, 3 * Kb, 0, [(1, M)]),
                                             rt[:, x0:x0 + nxw],
                                             start=True, stop=True)
                        ot = pout.tile([M, 2 * NX], odt, tag="cout", name="cout")
                        epi(ps, ot, g)
                        weng = nc.scalar if g % 2 == 0 else nc.gpsimd
                        weng.dma_start(out=dst_ap(g), in_=ot[:])

                # conv1: img -> act1  (K=30=(3cg,10yi), M=128=(8yo,16c))
                conv(prhs, ppsum, pout, wtl["w1"], 30, 128, 1, 48,
                     simple_load(img, 3, 10, 8, 3),
                     lambda g: dap(act1, (1 + 8 * g) * 16 * WP + 1,
                                   [(WP, 128), (1, 640)]),
                     bias_relu(b1t, 128), bf16, nc.scalar)
                # conv2: act1 -> act2  (K=128=(16cg,8yi), M=96=(6yo,16c)), with
                # convd slices interleaved so convd's DMAs hide in conv2's slack
                def conv2_chunk(gr):
                    conv(prhs, ppsum, pout, wtl["w2"], 128, 96, 1, 64,
                         simple_load(act1, 16, 8, 6, 16),
                         lambda g: dap(act2, (1 + 6 * g) * 16 * WP + 1,
                                       [(WP, 96), (1, 640)]),
                         bias_relu(b2t, 96), bf16, nc.scalar, gr=gr)

                def convd_chunk(gr):
                    convf(wtl["wd"], 18, 128, 24, 16, 18, 1, dpad, 1,
                          lambda g: dap(dfeat, (16 * g) * 8 * W, [(W, 128), (1, 640)]),
                          copy_epi(128), bf16, gr=gr)

                for ck in range(4):
                    conv2_chunk(range(16 * ck, 16 * ck + 16))
                    convd_chunk(range(6 * ck, 6 * ck + 6))
                # conv3: act2 -> act3
                conv(prhs, ppsum, pout, wtl["w3"], 128, 96, 1, 64,
                     simple_load(act2, 16, 8, 6, 16),
                     lambda g: dap(act3, (1 + 6 * g) * 16 * WP + 1,
                                   [(WP, 96), (1, 640)]),
                     bias_relu(b3t, 96), bf16, nc.scalar)
                # conv4: act3 -> Gt (K=112=(16cg,7yi), M=120=(5yo,24c))
                conv(prhs, ppsum, pout, wtl["w4"], 112, 120, 1, 77,
                     simple_load(act3, 16, 7, 5, 16),
                     lambda g: dap(Gt, (5 * g) * 24 * W, [(W, 120), (1, 640)]),
                     copy_epi(120), bf16, nc.scalar)

            # ---------------- scan-resident gate/w0 tiles + normalization
            with tc.tile_pool(name="scanbig", bufs=1) as pbig:
                # gw tap-fastest [24 rows, W cols, 3 taps]; pf column-major
                # [WP cols, 26 rows] -- every scan AP is then innermost-
                # contiguous, which unlocks the DVE 2x bf16 mode for the
                # per-step 3-tap multiply.
                gw = pbig.tile([128, 24, W, 3], bf16, tag="gw")
                gwv = gw[:]
                w0w = pbig.tile([128, 24, W], bf16, tag="w0w")
                w0v = w0w[:]
                TB = 8  # w0-staging chunk
                pf = pbig.tile([128, WP, 26], bf16, tag="pf")
                pfv = pf[:]
                # only col 0 (initial state + first-step halos) must be zero;
                # every other cell is written before it is read.
                nc.vector.memset(sap(pfv, 128, 0, [(1, 26)]), 0.0)
                prw = pbig.tile([128, 12, TB, 24, 4], bf16, tag="prw")
                prv = prw[:]

                # normalization: per output row r (24 rows per h-block), partitions
                # are (c:8, hb:16); Gt row-interleaved [385,24,W] gives 3-dim APs.
                with (tc.tile_pool(name="norm3", bufs=6) as pn3,
                      tc.tile_pool(name="norm1", bufs=2) as pn1,
                      tc.tile_pool(name="normf", bufs=3) as pnf):
                    for r in range(24):
                        gl = []
                        for tap in range(3):
                            g2 = pn3.tile([128, W], bf16, tag="gld", name="gld")
                            nc.sync.dma_start(
                                out=g2[:],
                                in_=dap(Gt, r * 24 * W + tap * 8 * W,
                                        [(W, 8), (24 * 24 * W, 16), (1, W)]))
                            gl.append(g2)
                        df = pnf.tile([128, W], bf16, tag="dfl", name="dfl")
                        nc.sync.dma_start(
                            out=df[:],
                            in_=dap(dfeat, r * 8 * W, [(W, 8), (24 * 8 * W, 16), (1, W)]))
                        ab = []
                        for tap in range(3):
                            a = pn3.tile([128, W], bf16, tag="gabs", name="gabs")
                            # |G2 + 1e-8| != |G2| only within 1e-8; keeps the
                            # div-by-zero guard without a separate epsilon op
                            # (1e-8 is not bf16-representable, so G2+1e-8 can
                            # never cancel to exactly 0).
                            nc.scalar.activation(a[:], gl[tap][:], ACTF.Abs,
                                                 bias=epst[:] if tap == 2 else 0.0)
                            ab.append(a)
                        s12 = pn1.tile([128, W], bf16, tag="s12")
                        nc.vector.tensor_tensor(out=s12[:], in0=ab[0][:], in1=ab[1][:],
                                                op=ALU.add)
                        sf = pnf.tile([128, W], f32, tag="sf")
                        nc.gpsimd.tensor_tensor(out=sf[:], in0=ab[2][:], in1=s12[:],
                                                op=ALU.add)
                        nc.vector.reciprocal(sf[:], sf[:])
                        for tap, eng in zip(range(3), (nc.gpsimd, nc.gpsimd, nc.vector)):
                            o = sap(gwv, 128, r * (W * 3) + tap, [(3, W)])
                            eng.tensor_tensor(out=o, in0=gl[tap][:], in1=sf[:],
                                              op=ALU.mult)
                        # t2 must come from the ROUNDED bf16 gates so the scan
                        # coefficients sum to exactly 1 (else drift accumulates
                        # over the 640-step scan).
                        gs0 = sap(gwv, 128, r * (W * 3) + 0, [(3, W)])
                        gs1 = sap(gwv, 128, r * (W * 3) + 1, [(3, W)])
                        gs2 = sap(gwv, 128, r * (W * 3) + 2, [(3, W)])
                        a12 = pn1.tile([128, W], bf16, tag="a12")
                        nc.gpsimd.tensor_tensor(out=a12[:], in0=gs0, in1=gs1,
                                                op=ALU.add)
                        nc.vector.tensor_tensor(out=a12[:], in0=a12[:], in1=gs2,
                                                op=ALU.add)
                        nc.vector.tensor_scalar(a12[:], a12[:], -1.0, 1.0,
                                                ALU.mult, ALU.add)
                        nc.gpsimd.tensor_tensor(out=sap(w0v, 128, r * W, [(1, W)]),
                                                in0=a12[:], in1=df[:], op=ALU.mult)
                    # zero edge gates: tap0 (up) row 0 of hb=0; tap2 (dn) row 23 of hb=15
                    pstep = gwv.ap[0][0]
                    nc.sync.dma_start(
                        out=bass.AP(gwv.tensor, gwv.offset,
                                    [[16 * pstep, 8], [3, W], [1, 1]]),
                        in_=sap(ztv, 8, 0, [(1, W), (0, 1)]))
                    for c_ in range(8):
                        nc.sync.dma_start(
                            out=bass.AP(gwv.tensor,
                                        gwv.offset + (16 * c_ + 15) * pstep
                                        + 23 * (W * 3) + 2,
                                        [[pstep, 1], [3, W], [1, 1]]),
                            in_=sap(ztv, 1, 0, [(1, W), (0, 1)]))

                # ---------------- SPN scan (pf column-major: elem (x, r) at x*26+r)
                mask_up = [(i - 1) % 32 for i in range(32)]
                mask_dn = [(i + 1) % 32 for i in range(32)]
                for t in range(W):
                    j = t % TB
                    bi = (t // TB) % 12
                    if j == 0:
                        nc.scalar.copy(
                            sap(prv, 128, bi * (TB * 96) + 3, [(4, 24), (96, TB)]),
                            sap(w0v, 128, t, [(W, 24), (1, TB)]))
                    base = bi * (TB * 96) + j * 96
                    taps = sap(pfv, 128, t * 26, [(1, 24), (1, 3)])
                    g_t = sap(gwv, 128, t * 3, [(3 * W, 24), (1, 3)])
                    nc.vector.tensor_tensor(out=sap(prv, 128, base, [(4, 24), (1, 3)]),
                                            in0=g_t, in1=taps, op=ALU.mult)
                    with nc.allow_low_precision(reason="bf16 scan state, validated"):
                        nc.vector.tensor_reduce(out=sap(pfv, 128, (1 + t) * 26 + 1, [(1, 24)]),
                                                in_=sap(prv, 128, base, [(4, 24), (1, 4)]),
                                                axis=AX.X, op=ALU.add)
                    nc.vector.stream_shuffle(out=sap(pfv, 128, (1 + t) * 26, [(1, 1)]),
                                             in_=sap(pfv, 128, (1 + t) * 26 + 24, [(1, 1)]),
                                             mask=mask_up)
                    nc.vector.stream_shuffle(out=sap(pfv, 128, (1 + t) * 26 + 25, [(1, 1)]),
                                             in_=sap(pfv, 128, (1 + t) * 26 + 1, [(1, 1)]),
                                             mask=mask_dn)
                # ---------------- prop export + convc, split into column halves:
                # half A (out cols 0..319) depends only on scan steps <= 321,
                # so its exports/loads/matmuls/epilogue all run during the
                # scan's second half on the otherwise-idle engines.
                with (tc.tile_pool(name="rhsc", bufs=8) as prhs2,
                      tc.tile_pool(name="coutc", bufs=8) as pout2,
                      tc.tile_pool(name="psumc", bufs=4, space="PSUM") as ppsum2):
                    wcv = wtl["wc"][:]

                    def convc_part(xo, xw, ex0, exn, in_scan):
                        # export data cols ex0..ex0+exn-1 (prop col = x+1).
                        # While the scan is live, Act runs ONLY the relus (so
                        # w0-staging copies on Act are never starved); exports
                        # go to SP, dtile/writes to Pool.
                        nw = xw + 2
                        with nc.allow_non_contiguous_dma(reason="pf col-major"):
                            for r in range(24):
                                eng = nc.sync if (in_scan or r % 2 == 0) else nc.scalar
                                eng.dma_start(
                                    out=dap(prop, r * WP + 1 + ex0,
                                            [(24 * WP, 128), (1, exn)]),
                                    in_=sap(pfv, 128, (1 + ex0) * 26 + 1 + r,
                                            [(26, exn), (1, 1)]))
                        c0 = xo  # first prop col in rhs tiles
                        for g in range(24):
                            rhss = []
                            for p_ in range(2):
                                ld = nc.sync if (in_scan or p_ == 0) else nc.scalar
                                rt = prhs2.tile([72, nw], bf16, tag="rhs", name="rhsc")
                                rv = rt[:]
                                pstep = rv.ap[0][0]
                                if g == 0 or g == 23:
                                    zrow = 0 if g == 0 else 17
                                    drow0, prow0 = (1, 0) if g == 0 else (0, 367)
                                    for cg in range(4):
                                        nc.gpsimd.dma_start(
                                            out=bass.AP(rv.tensor,
                                                        rv.offset + (cg * 18 + zrow) * pstep,
                                                        [[pstep, 1], [1, nw]]),
                                            in_=sap(ztv, 1, 0, [(1, nw)]))
                                        ld.dma_start(
                                            out=bass.AP(rv.tensor,
                                                        rv.offset + (cg * 18 + drow0) * pstep,
                                                        [[pstep, 17], [1, nw]]),
                                            in_=dap(prop,
                                                    (p_ * 4 + cg) * H * WP
                                                    + prow0 * WP + c0,
                                                    [(WP, 17), (1, nw)]))
                                else:
                                    ld.dma_start(
                                        out=rv,
                                        in_=dap(prop,
                                                p_ * 4 * H * WP + (16 * g - 1) * WP + c0,
                                                [(H * WP, 4), (WP, 18), (1, nw)]))
                                rhss.append(rt)
                            # disp residual enters PSUM via an identity matmul,
                            # so the epilogue is a single Act relu.
                            dt_ = pout2.tile([16, xw], bf16, tag="dtile", name="dtile")
                            nc.gpsimd.dma_start(
                                out=dt_[:],
                                in_=dap(dispb, (16 * g) * W + xo, [(W, 16), (1, xw)]))
                            ps = ppsum2.tile([16, 512], f32, tag="psc", name="psc")
                            for p_ in range(2):
                                for dx in range(3):
                                    nc.tensor.matmul(
                                        ps[:, :xw],
                                        sap(wcv, 72, (p_ * 3 + dx) * 16, [(1, 16)]),
                                        rhss[p_][:, dx:dx + xw],
                                        start=(p_ == 0 and dx == 0),
                                        stop=False)
                            nc.tensor.matmul(ps[:, :xw], idt[:], dt_[:],
                                             start=False, stop=True)
                            ot = pout2.tile([16, xw], f32, tag="cout", name="coutc")
                            if not in_scan and g % 2 == 0:
                                nc.vector.tensor_scalar(ot[:], ps[:, :xw], 0.0, None,
                                                        ALU.max)
                            else:
                                nc.scalar.activation(ot[:], ps[:, :xw], ACTF.Relu)
                            if in_scan:
                                weng = nc.gpsimd
                            else:
                                weng = nc.sync if g % 2 == 0 else nc.gpsimd
                            weng.dma_start(
                                out=dap(outp, (16 * g) * W + xo, [(W, 16), (1, xw)]),
                                in_=ot[:])

                    convc_part(0, 240, 0, 241, True)
                    convc_part(240, 240, 241, 240, True)
                    convc_part(480, 160, 481, 159, False)

    # Engine-sem update thinning: Tile increments each engine's sem on every
    # op, but only values that some wait references matter. Dropping the rest
    # (and renumbering waits to the kept-update count at the same producer op)
    # is semantically exact and shrinks the sem-update backlog.
    ENG_SEMS = ("DVE_44", "PE_44", "Activation_44", "Pool_44", "SP_44")
    insts_all = []
    for fn in nc.m.functions:
        for bb in fn.blocks:
            insts_all.extend(bb.instructions)
    waited = {sm: set() for sm in ENG_SEMS}
    for inst in insts_all:
        si = inst.sync_info
        if si and si.on_wait:
            for wt_ in si.on_wait:
                if wt_.ant_name in waited:
                    waited[wt_.ant_name].add(wt_.wait_value)
    cum = {sm: 0 for sm in ENG_SEMS}
    newcum = {sm: 0 for sm in ENG_SEMS}
    remap = {sm: {} for sm in ENG_SEMS}
    for inst in insts_all:
        si = inst.sync_info
        if not si:
            continue
        ups = si.on_update
        if ups:
            keep = []
            for u in ups:
                sm = u.ant_name
                if sm in cum:
                    cum[sm] += 1
                    if cum[sm] in waited[sm]:
                        newcum[sm] += 1
                        remap[sm][cum[sm]] = newcum[sm]
                        keep.append(u)
                    # else: drop this update
                else:
                    keep.append(u)
            if len(keep) != len(ups):
                si.on_update = keep
    for inst in insts_all:
        si = inst.sync_info
        if si and si.on_wait:
            ws = list(si.on_wait)
            ch = False
            for i_, wt_ in enumerate(ws):
                if wt_.ant_name in remap and wt_.wait_value in remap[wt_.ant_name]:
                    nv = remap[wt_.ant_name][wt_.wait_value]
                    if nv != wt_.wait_value:
                        wt_.wait_value = nv
                        ch = True
            if ch:
                si.on_wait = ws

    # walrus on this stack accepts at most one sync wait per instruction:
    # spill excess waits onto same-engine NOPs injected just before.
    nwn = [0]
    for fn in nc.m.functions:
        for bb in fn.blocks:
            il = bb.instructions
            i = 0
            while i < len(il):
                inst = il[i]
                si = inst.sync_info
                if si is not None and si.on_wait and len(si.on_wait) > 1:
                    waits = list(si.on_wait)
                    si.on_wait = waits[-1:]
                    for j in range(len(waits) - 1):
                        nwn[0] += 1
                        nop = mybir.InstNoOp(name=f"WS-{nwn[0]}", ins=[], outs=[])
                        nop.engine = inst.engine
                        nop.sync_info = mybir.SyncInfo(on_wait=[waits[j]], on_update=[])
                        nc.register_instruction(nop, overwrite=True)
                        il.insert(i, nop)
                        i += 1
                i += 1

    return nc


def _prep_inputs(inputs):
    w1, b1 = _fold_bn(inputs['w1'], inputs['bn1_g'], inputs['bn1_b'], inputs['bn1_m'], inputs['bn1_v'])
    w2, b2 = _fold_bn(inputs['w2'], inputs['bn2_g'], inputs['bn2_b'], inputs['bn2_m'], inputs['bn2_v'])
    w3, b3 = _fold_bn(inputs['w3'], inputs['bn3_g'], inputs['bn3_b'], inputs['bn3_m'], inputs['bn3_v'])

    w1k = _lhsT(w1, 8, 10, 3, 1).astype(BF)                       # [30, 384]
    w2k = _lhsT(w2, 6, 8, 16, 1).astype(BF)                       # [128, 288]
    w3k = _lhsT(w3, 6, 8, 16, 1).astype(BF)
    w4k = _lhsT(inputs['w4'].astype(np.float32), 5, 7, 16, 1).astype(BF)   # [112, 360]
    wdk = _lhsT_dx(inputs['wd'].astype(np.float32), 16, 18).astype(BF)     # [54, 128]
    wck = _lhsT(inputs['wc'].astype(np.float32), 16, 18, 4, 2).astype(BF)  # [72, 96]

    b1r = np.tile(b1, 8).reshape(128, 1).astype(np.float32)
    b2r = np.tile(b2, 6).reshape(96, 1).astype(np.float32)
    b3r = np.tile(b3, 6).reshape(96, 1).astype(np.float32)

    maps = []
    for b in range(8):
        maps.append({
            "img": _pad_rowi(inputs['leftImage'][b]),
            "dpad": _pad_rowi(inputs['disp'][b]),
            "dispb": inputs['disp'][b, 0].astype(BF),
            "w1k": w1k, "w2k": w2k, "w3k": w3k, "w4k": w4k, "wdk": wdk, "wck": wck,
            "b1v": b1r, "b2v": b2r, "b3v": b3r, "idv": np.eye(16, dtype=BF),
        })
    return maps


def kernel(**inputs):
    from concourse.bass_utils import run_bass_kernel_spmd

    if "nc" not in _CACHE:
        _CACHE["nc"] = _build()
    nc = _CACHE["nc"]
    maps = _prep_inputs(inputs)
    res = run_bass_kernel_spmd(nc, maps, core_ids=list(range(8)))
    out = np.stack([res.results[i]["out"] for i in range(8)])[:, None].astype(np.float32)
    return out


# revision 80
# speedup vs baseline: 3.5273x; 1.0067x over previous
"""Trainium2 Bass kernel for nn_AnyNetRefinement (disparity refinement with SPN scan).

Data-parallel over batch: core b processes image b end-to-end (no collectives).

v2: all DRAM round-trips use row-interleaved layouts ([row, channel, col]) with
matmul M-order (row, channel), so every conv load/store DMA collapses to a
2-dim AP [(WP, M), (1, cols)] (single-descriptor-chain cost). Epilogues on
gpsimd, writes on scalar queue, loads on sync queue. Gate normalization is
chunked per output row and balanced across Act/Pool/DVE. The SPN scan is the
folded [128=(c,hblock), 26] bf16 state machine (3-tap multiply + 4-slot
reduce + halo shuffles). Prop is exported to a channel-planar DRAM buffer in
24 merged per-block-row DMAs and convc consumes it directly.
"""

import numpy as np
import ml_dtypes

BF = ml_dtypes.bfloat16

H, W = 384, 640
HP, WP = 387, 642        # padded rows (+1 top, +2 bottom), cols (+1 left, +1 right)
NX = 320                 # half free width (psum-bank safe: 512+128 split)

_CACHE = {}


# ---------------------------------------------------------------- host helpers
def _fold_bn(wt, g, b, m, v):
    s = g / np.sqrt(v + 1e-5)
    return (wt * s.reshape(-1, 1, 1, 1)).astype(np.float32), (b - m * s).astype(np.float32)


def _lhsT(wt, r_out, r_in, cin_g, npass):
    """lhsT [K=(cin_g,yi), npass, 3, M=(yo,cout)] -- M row-major."""
    cout, cin = wt.shape[0], wt.shape[1]
    K = cin_g * r_in
    M = cout * r_out
    out = np.zeros((K, npass, 3, M), np.float32)
    for p in range(npass):
        for cg in range(cin_g):
            c = p * cin_g + cg
            if c >= cin:
                continue
            for dx in range(3):
                for yi in range(r_in):
                    k = cg * r_in + yi
                    for co in range(cout):
                        for yo in range(r_out):
                            dy = yi - yo
                            if 0 <= dy <= 2:
                                out[k, p, dx, yo * cout + co] = wt[co, c, dy, dx]
    return out.reshape(K, npass * 3 * M)


def _lhsT_dx(wt, r_out, r_in):
    """dx folded into K: lhsT [K=(dx,cin,yi), M=(yo,cout)] (single matmul/half)."""
    cout, cin = wt.shape[0], wt.shape[1]
    Kb = cin * r_in
    M = cout * r_out
    out = np.zeros((3 * Kb, M), np.float32)
    for dx in range(3):
        for cg in range(cin):
            for yi in range(r_in):
                k = dx * Kb + cg * r_in + yi
                for co in range(cout):
                    for yo in range(r_out):
                        dy = yi - yo
                        if 0 <= dy <= 2:
                            out[k, yo * cout + co] = wt[co, cg, dy, dx]
    return out


def _pad_rowi(x):
    """[C, H, W] -> row-interleaved padded [HP, C, WP] bf16."""
    c = x.shape[0]
    out = np.zeros((HP, c, WP), BF)
    out[1:1 + H, :, 1:1 + W] = np.moveaxis(x, 0, 1).astype(BF)
    return out


# ---------------------------------------------------------------- bass builder
def _build():
    import concourse.bass as bass
    import concourse.mybir as mybir
    from concourse import tile
    from concourse.vector_clock import ScopedClock

    f32 = mybir.dt.float32
    bf16 = mybir.dt.bfloat16
    ALU = mybir.AluOpType
    ACTF = mybir.ActivationFunctionType
    AX = mybir.AxisListType

    class TC(tile.TileContext):
        # this walrus build accepts only one sync-wait per Drain; split the
        # end-of-kernel waits across several drains.
        def _drain_and_barrier(self, tick_clock, wait_clock):
            nc = self.nc
            drain_inst = nc.sync.drain()
            wait_clock.add_sem_waits(drain_inst.ins, ScopedClock({None: tick_clock.global_clock}))
            waits = list(drain_inst.ins.sync_info.on_wait)
            if len(waits) > 1:
                drain_inst.ins.sync_info.on_wait = waits[:1]
                for i in range(1, len(waits)):
                    d2 = nc.sync.drain()
                    if d2.ins.sync_info is None:
                        d2.ins.sync_info = mybir.SyncInfo(on_wait=[waits[i]], on_update=[])
                    else:
                        d2.ins.sync_info.on_wait = [waits[i]]
            nc.all_engine_barrier()
            popped = nc._tile_sem_poison_stack.pop()
            assert popped is self._sem_poison
            nc.clear_and_free_semaphores(list(self.sems.allocated().values()))
            nc.all_engine_barrier()

    def dap(t, offset, dims):
        base = t if isinstance(t, bass.AP) else t[:]
        return bass.AP(base.tensor, base.offset + offset, [list(d) for d in dims])

    def sap(tile_ap, nparts, offset, dims, pstride=1):
        pstep = tile_ap.ap[0][0]
        return bass.AP(tile_ap.tensor, tile_ap.offset + offset,
                       [[pstep * pstride, nparts]] + [list(d) for d in dims])

    nc = bass.Bass("TRN2", num_swdge_queues=4)

    img = nc.declare_dram_parameter("img", [HP, 3, WP], bf16, isOutput=False)
    dpad = nc.declare_dram_parameter("dpad", [HP, 1, WP], bf16, isOutput=False)
    dispb = nc.declare_dram_parameter("dispb", [H, W], bf16, isOutput=False)
    w1k = nc.declare_dram_parameter("w1k", [30, 3 * 128], bf16, isOutput=False)
    w2k = nc.declare_dram_parameter("w2k", [128, 3 * 96], bf16, isOutput=False)
    w3k = nc.declare_dram_parameter("w3k", [128, 3 * 96], bf16, isOutput=False)
    w4k = nc.declare_dram_parameter("w4k", [112, 3 * 120], bf16, isOutput=False)
    wdk = nc.declare_dram_parameter("wdk", [54, 128], bf16, isOutput=False)
    wck = nc.declare_dram_parameter("wck", [72, 2 * 3 * 16], bf16, isOutput=False)
    b1v = nc.declare_dram_parameter("b1v", [128, 1], f32, isOutput=False)
    b2v = nc.declare_dram_parameter("b2v", [96, 1], f32, isOutput=False)
    b3v = nc.declare_dram_parameter("b3v", [96, 1], f32, isOutput=False)
    idv = nc.declare_dram_parameter("idv", [16, 16], bf16, isOutput=False)
    outp = nc.declare_dram_parameter("out", [H, W], f32, isOutput=True)

    with TC(nc) as tc:
        with (tc.tile_pool(name="dram", bufs=1, space="DRAM") as dram,
              tc.tile_pool(name="wts", bufs=1) as pw):
            act1 = dram.tile([HP, 16, WP], bf16, tag="act1")
            act2 = dram.tile([HP, 16, WP], bf16, tag="act2")
            act3 = dram.tile([HP, 16, WP], bf16, tag="act3")
            Gt = dram.tile([385, 24, W], bf16, tag="G")
            dfeat = dram.tile([H, 8, W], bf16, tag="dfeat")
            prop = dram.tile([8, H, WP], bf16, tag="prop")

            # ---------------- weights/biases
            wtl = {}
            wengs = (nc.sync, nc.scalar, nc.gpsimd)
            for wi, (nm, prm, kk, nm3) in enumerate(
                    (("w1", w1k, 30, 3 * 128), ("w2", w2k, 128, 3 * 96),
                     ("w3", w3k, 128, 3 * 96), ("w4", w4k, 112, 3 * 120),
                     ("wd", wdk, 54, 128), ("wc", wck, 72, 6 * 16))):
                t = pw.tile([kk, nm3], bf16, tag=f"{nm}t", name=f"{nm}t")
                wengs[wi % 3].dma_start(out=t[:], in_=prm[:])
                wtl[nm] = t
            b1t = pw.tile([128, 1], f32, tag="b1t")
            nc.scalar.dma_start(out=b1t[:], in_=b1v[:])
            b2t = pw.tile([96, 1], f32, tag="b2t")
            nc.gpsimd.dma_start(out=b2t[:], in_=b2v[:])
            b3t = pw.tile([96, 1], f32, tag="b3t")
            nc.scalar.dma_start(out=b3t[:], in_=b3v[:])
            idt = pw.tile([16, 16], bf16, tag="idt")
            nc.gpsimd.dma_start(out=idt[:], in_=idv[:])
            epst = pw.tile([128, 1], f32, tag="epst")
            nc.gpsimd.memset(epst[:], 1e-8)

            # ---------------- zero borders of padded DRAM buffers
            zt = pw.tile([128, WP], bf16, tag="zt")
            nc.vector.memset(zt[:], 0.0)
            ztv = zt[:]
            for buf in (act1, act2, act3):
                # row 0 (16ch) and rows 385,386 (32 rows*ch), full padded width
                nc.gpsimd.dma_start(out=dap(buf, 0, [(WP, 16), (1, WP)]),
                                    in_=sap(ztv, 16, 0, [(1, WP)]))
                nc.gpsimd.dma_start(out=dap(buf, 385 * 16 * WP, [(WP, 32), (1, WP)]),
                                    in_=sap(ztv, 32, 0, [(1, WP)]))
                # cols 0 and WP-1, rows 1..384 all ch (merged (row,ch) stride WP)
                for col in (0, WP - 1):
                    nc.gpsimd.dma_start(
                        out=dap(buf, 16 * WP + col, [(WP, 16 * H), (1, 1)]),
                        in_=sap(ztv, 128, 0, [(0, 48), (1, 1)]))
            # prop: cols 0/641, all rows*channels (merged stride WP)
            for col in (0, WP - 1):
                nc.gpsimd.dma_start(
                    out=dap(prop, col, [(WP, 8 * H), (1, 1)]),
                    in_=sap(ztv, 128, 0, [(0, 24), (1, 1)]))

            # ---------------- generic conv (row-group pipelined)
            def conv(prhs, ppsum, pout, wt, K, M, npass, S, load, dst_ap, epi,
                     odt, st, gr=None):
                wv = wt[:]
                for g in (gr if gr is not None else range(S)):
                    rhss = []
                    for p_ in range(npass):
                        rt = prhs.tile([K, WP], bf16, tag="rhs", name="rhs")
                        load(rt, p_, g)
                        rhss.append(rt)
                    ps = ppsum.tile([128, 2 * NX], f32, tag="ps", name="ps")
                    for x0, nxw in ((0, 512), (512, 128)):
                        for p_ in range(npass):
                            for dx in range(3):
                                nc.tensor.matmul(
                                    ps[:M, x0:x0 + nxw],
                                    sap(wv, K, (p_ * 3 + dx) * M, [(1, M)]),
                                    rhss[p_][:, x0 + dx:x0 + dx + nxw],
                                    start=(p_ == 0 and dx == 0),
                                    stop=(p_ == npass - 1 and dx == 2))
                    ot = pout.tile([M, 2 * NX], odt, tag="cout", name="cout")
                    epi(ps, ot, g)
                    weng = st if g % 2 == 0 else nc.gpsimd
                    weng.dma_start(out=dst_ap(g), in_=ot[:])

            with (tc.tile_pool(name="rhs", bufs=12) as prhs,
                  tc.tile_pool(name="cout", bufs=12) as pout,
                  tc.tile_pool(name="psum", bufs=4, space="PSUM") as ppsum):
                # NOTE: gpsimd cannot touch PSUM (BIR verifier) -- epilogues
                # that read PSUM live on DVE (tensor_scalar) or Act (activation
                # relu-with-bias), split 2:1 to keep both under PE.
                def bias_relu(bt, M):
                    def epi(ps, ot, g):
                        if g % 3 == 2:
                            nc.scalar.activation(ot[:], ps[:M, :], ACTF.Relu,
                                                 bias=bt[:M, :])
                        else:
                            nc.vector.tensor_scalar(ot[:], ps[:M, :], bt[:M, :], 0.0,
                                                    ALU.add, ALU.max)
                    return epi

                def copy_epi(M):
                    def epi(ps, ot, g):
                        if g % 3 == 2:
                            nc.scalar.activation(ot[:], ps[:M, :], ACTF.Copy)
                        else:
                            nc.vector.tensor_copy(ot[:], ps[:M, :])
                    return epi

                def simple_load(src, cg_n, rin, r, cpitch):
                    def load(rt, p_, g):
                        nc.sync.dma_start(
                            out=rt[:],
                            in_=dap(src, (r * g) * cpitch * WP,
                                    [(WP, cg_n), (cpitch * WP, rin), (1, WP)])
                            if cg_n > 1 else
                            dap(src, (r * g) * WP, [(WP, rin), (1, WP)]))
                    return load

                # dx folded into K (3 shifted rhs copies): one matmul per half
                def convf(wt, Kb, M, S, r, rin, cin, src, cpitch, dst_ap, epi, odt,
                          gr=None):
                    wv = wt[:]
                    engs = (nc.sync, nc.scalar, nc.gpsimd)
                    for g in (gr if gr is not None else range(S)):
                        rt = prhs.tile([3 * Kb, 640], bf16, tag="rhs", name="rhsf")
                        rv = rt[:]
                        pstep = rv.ap[0][0]
                        for dx in range(3):
                            engs[dx].dma_start(
                                out=bass.AP(rv.tensor, rv.offset + (dx * Kb) * pstep,
                                            [[pstep, Kb], [1, 640]]),
                                in_=dap(src, (r * g) * cpitch * WP + dx,
                                        [(WP, cin), (cpitch * WP, rin), (1, 640)])
                                if cin > 1 else
                                dap(src, (r * g) * WP + dx, [(WP, rin), (1, 640)]))
                        ps = ppsum.tile([128, 2 * NX], f32, tag="ps", name="ps")
                        for x0, nxw in ((0, 512), (512, 128)):
                            nc.tensor.matmul(ps[:M, x0:x0 + nxw],
                                             sap(wv, 3 * Kb, 0, [(1, M)]),
                                             rt[:, x0:x0 + nxw],
                                             start=True, stop=True)
                        ot = pout.tile([M, 2 * NX], odt, tag="cout", name="cout")
                        epi(ps, ot, g)
                        weng = nc.scalar if g % 2 == 0 else nc.gpsimd
                        weng.dma_start(out=dst_ap(g), in_=ot[:])

                # conv1: img -> act1  (K=30=(3cg,10yi), M=128=(8yo,16c))
                conv(prhs, ppsum, pout, wtl["w1"], 30, 128, 1, 48,
                     simple_load(img, 3, 10, 8, 3),
                     lambda g: dap(act1, (1 + 8 * g) * 16 * WP + 1,
                                   [(WP, 128), (1, 640)]),
                     bias_relu(b1t, 128), bf16, nc.scalar)
                # conv2: act1 -> act2  (K=128=(16cg,8yi), M=96=(6yo,16c)), with
                # convd slices interleaved so convd's DMAs hide in conv2's slack
                def conv2_chunk(gr):
                    conv(prhs, ppsum, pout, wtl["w2"], 128, 96, 1, 64,
                         simple_load(act1, 16, 8, 6, 16),
                         lambda g: dap(act2, (1 + 6 * g) * 16 * WP + 1,
                                       [(WP, 96), (1, 640)]),
                         bias_relu(b2t, 96), bf16, nc.scalar, gr=gr)

                def convd_chunk(gr):
                    convf(wtl["wd"], 18, 128, 24, 16, 18, 1, dpad, 1,
                          lambda g: dap(dfeat, (16 * g) * 8 * W, [(W, 128), (1, 640)]),
                          copy_epi(128), bf16, gr=gr)

                for ck in range(4):
                    conv2_chunk(range(16 * ck, 16 * ck + 16))
                    convd_chunk(range(6 * ck, 6 * ck + 6))
                # conv3: act2 -> act3
                conv(prhs, ppsum, pout, wtl["w3"], 128, 96, 1, 64,
                     simple_load(act2, 16, 8, 6, 16),
                     lambda g: dap(act3, (1 + 6 * g) * 16 * WP + 1,
                                   [(WP, 96), (1, 640)]),
                     bias_relu(b3t, 96), bf16, nc.scalar)
                # conv4: act3 -> Gt (K=112=(16cg,7yi), M=120=(5yo,24c))
                conv(prhs, ppsum, pout, wtl["w4"], 112, 120, 1, 77,
                     simple_load(act3, 16, 7, 5, 16),
                     lambda g: dap(Gt, (5 * g) * 24 * W, [(W, 120), (1, 640)]),
                     copy_epi(120), bf16, nc.scalar)

            # ---------------- scan-resident gate/w0 tiles + normalization
            with tc.tile_pool(name="scanbig", bufs=1) as pbig:
                # gw tap-fastest [24 rows, W cols, 3 taps]; pf column-major
                # [WP cols, 26 rows] -- every scan AP is then innermost-
                # contiguous, which unlocks the DVE 2x bf16 mode for the
                # per-step 3-tap multiply.
                gw = pbig.tile([128, 24, W, 3], bf16, tag="gw")
                gwv = gw[:]
                w0w = pbig.tile([128, 24, W], bf16, tag="w0w")
                w0v = w0w[:]
                TB = 8  # w0-staging chunk
                pf = pbig.tile([128, WP, 26], bf16, tag="pf")
                pfv = pf[:]
                # only col 0 (initial state + first-step halos) must be zero;
                # every other cell is written before it is read.
                nc.vector.memset(sap(pfv, 128, 0, [(1, 26)]), 0.0)
                prw = pbig.tile([128, 12, TB, 24, 4], bf16, tag="prw")
                prv = prw[:]

                # normalization: per output row r (24 rows per h-block), partitions
                # are (c:8, hb:16); Gt row-interleaved [385,24,W] gives 3-dim APs.
                with (tc.tile_pool(name="norm3", bufs=6) as pn3,
                      tc.tile_pool(name="norm1", bufs=2) as pn1,
                      tc.tile_pool(name="normf", bufs=3) as pnf):
                    for r in [0, 23] + [x for x in range(1, 23)]:
                        gl = []
                        for tap in range(3):
                            g2 = pn3.tile([128, W], bf16, tag="gld", name="gld")
                            nc.sync.dma_start(
                                out=g2[:],
                                in_=dap(Gt, r * 24 * W + tap * 8 * W,
                                        [(W, 8), (24 * 24 * W, 16), (1, W)]))
                            gl.append(g2)
                        df = pnf.tile([128, W], bf16, tag="dfl", name="dfl")
                        nc.sync.dma_start(
                            out=df[:],
                            in_=dap(dfeat, r * 8 * W, [(W, 8), (24 * 8 * W, 16), (1, W)]))
                        ab = []
                        for tap in range(3):
                            a = pn3.tile([128, W], bf16, tag="gabs", name="gabs")
                            # |G2 + 1e-8| != |G2| only within 1e-8; keeps the
                            # div-by-zero guard without a separate epsilon op
                            # (1e-8 is not bf16-representable, so G2+1e-8 can
                            # never cancel to exactly 0).
                            nc.scalar.activation(a[:], gl[tap][:], ACTF.Abs,
                                                 bias=epst[:] if tap == 2 else 0.0)
                            ab.append(a)
                        s12 = pn1.tile([128, W], bf16, tag="s12")
                        nc.vector.tensor_tensor(out=s12[:], in0=ab[0][:], in1=ab[1][:],
                                                op=ALU.add)
                        sf = pnf.tile([128, W], f32, tag="sf")
                        nc.gpsimd.tensor_tensor(out=sf[:], in0=ab[2][:], in1=s12[:],
                                                op=ALU.add)
                        nc.vector.reciprocal(sf[:], sf[:])
                        for tap, eng in zip(range(3), (nc.gpsimd, nc.gpsimd, nc.vector)):
                            o = sap(gwv, 128, r * (W * 3) + tap, [(3, W)])
                            eng.tensor_tensor(out=o, in0=gl[tap][:], in1=sf[:],
                                              op=ALU.mult)
                        # t2 must come from the ROUNDED bf16 gates so the scan
                        # coefficients sum to exactly 1 (else drift accumulates
                        # over the 640-step scan).
                        gs0 = sap(gwv, 128, r * (W * 3) + 0, [(3, W)])
                        gs1 = sap(gwv, 128, r * (W * 3) + 1, [(3, W)])
                        gs2 = sap(gwv, 128, r * (W * 3) + 2, [(3, W)])
                        a12 = pn1.tile([128, W], bf16, tag="a12")
                        nc.gpsimd.tensor_tensor(out=a12[:], in0=gs0, in1=gs1,
                                                op=ALU.add)
                        nc.vector.tensor_tensor(out=a12[:], in0=a12[:], in1=gs2,
                                                op=ALU.add)
                        nc.vector.tensor_scalar(a12[:], a12[:], -1.0, 1.0,
                                                ALU.mult, ALU.add)
                        nc.gpsimd.tensor_tensor(out=sap(w0v, 128, r * W, [(1, W)]),
                                                in0=a12[:], in1=df[:], op=ALU.mult)
                    # zero edge gates: tap0 (up) row 0 of hb=0; tap2 (dn) row 23 of hb=15
                    pstep = gwv.ap[0][0]
                    nc.sync.dma_start(
                        out=bass.AP(gwv.tensor, gwv.offset,
                                    [[16 * pstep, 8], [3, W], [1, 1]]),
                        in_=sap(ztv, 8, 0, [(1, W), (0, 1)]))
                    for c_ in range(8):
                        nc.sync.dma_start(
                            out=bass.AP(gwv.tensor,
                                        gwv.offset + (16 * c_ + 15) * pstep
                                        + 23 * (W * 3) + 2,
                                        [[pstep, 1], [3, W], [1, 1]]),
                            in_=sap(ztv, 1, 0, [(1, W), (0, 1)]))

                # ---------------- SPN scan (pf column-major: elem (x, r) at x*26+r)
                mask_up = [(i - 1) % 32 for i in range(32)]
                mask_dn = [(i + 1) % 32 for i in range(32)]
                for t in range(W):
                    j = t % TB
                    bi = (t // TB) % 12
                    if j == 0:
                        nc.scalar.copy(
                            sap(prv, 128, bi * (TB * 96) + 3, [(4, 24), (96, TB)]),
                            sap(w0v, 128, t, [(W, 24), (1, TB)]))
                    base = bi * (TB * 96) + j * 96
                    taps = sap(pfv, 128, t * 26, [(1, 24), (1, 3)])
                    g_t = sap(gwv, 128, t * 3, [(3 * W, 24), (1, 3)])
                    nc.vector.tensor_tensor(out=sap(prv, 128, base, [(4, 24), (1, 3)]),
                                            in0=g_t, in1=taps, op=ALU.mult)
                    with nc.allow_low_precision(reason="bf16 scan state, validated"):
                        nc.vector.tensor_reduce(out=sap(pfv, 128, (1 + t) * 26 + 1, [(1, 24)]),
                                                in_=sap(prv, 128, base, [(4, 24), (1, 4)]),
                                                axis=AX.X, op=ALU.add)
                    nc.vector.stream_shuffle(out=sap(pfv, 128, (1 + t) * 26, [(1, 1)]),
                                             in_=sap(pfv, 128, (1 + t) * 26 + 24, [(1, 1)]),
                                             mask=mask_up)
                    nc.vector.stream_shuffle(out=sap(pfv, 128, (1 + t) * 26 + 25, [(1, 1)]),
                                             in_=sap(pfv, 128, (1 + t) * 26 + 1, [(1, 1)]),
                                             mask=mask_dn)
                # ---------------- prop export + convc, split into column halves:
                # half A (out cols 0..319) depends only on scan steps <= 321,
                # so its exports/loads/matmuls/epilogue all run during the
                # scan's second half on the otherwise-idle engines.
                with (tc.tile_pool(name="rhsc", bufs=8) as prhs2,
                      tc.tile_pool(name="coutc", bufs=8) as pout2,
                      tc.tile_pool(name="psumc", bufs=4, space="PSUM") as ppsum2):
                    wcv = wtl["wc"][:]

                    def convc_part(xo, xw, ex0, exn, in_scan):
                        # export data cols ex0..ex0+exn-1 (prop col = x+1).
                        # While the scan is live, Act runs ONLY the relus (so
                        # w0-staging copies on Act are never starved); exports
                        # go to SP, dtile/writes to Pool.
                        nw = xw + 2
                        with nc.allow_non_contiguous_dma(reason="pf col-major"):
                            for r in range(24):
                                eng = nc.sync if (in_scan or r % 2 == 0) else nc.scalar
                                eng.dma_start(
                                    out=dap(prop, r * WP + 1 + ex0,
                                            [(24 * WP, 128), (1, exn)]),
                                    in_=sap(pfv, 128, (1 + ex0) * 26 + 1 + r,
                                            [(26, exn), (1, 1)]))
                        c0 = xo  # first prop col in rhs tiles
                        for g in range(24):
                            rhss = []
                            for p_ in range(2):
                                ld = nc.sync if (in_scan or p_ == 0) else nc.scalar
                                rt = prhs2.tile([72, nw], bf16, tag="rhs", name="rhsc")
                                rv = rt[:]
                                pstep = rv.ap[0][0]
                                if g == 0 or g == 23:
                                    zrow = 0 if g == 0 else 17
                                    drow0, prow0 = (1, 0) if g == 0 else (0, 367)
                                    for cg in range(4):
                                        nc.gpsimd.dma_start(
                                            out=bass.AP(rv.tensor,
                                                        rv.offset + (cg * 18 + zrow) * pstep,
                                                        [[pstep, 1], [1, nw]]),
                                            in_=sap(ztv, 1, 0, [(1, nw)]))
                                        ld.dma_start(
                                            out=bass.AP(rv.tensor,
                                                        rv.offset + (cg * 18 + drow0) * pstep,
                                                        [[pstep, 17], [1, nw]]),
                                            in_=dap(prop,
                                                    (p_ * 4 + cg) * H * WP
                                                    + prow0 * WP + c0,
                                                    [(WP, 17), (1, nw)]))
                                else:
                                    ld.dma_start(
                                        out=rv,
                                        in_=dap(prop,
                                                p_ * 4 * H * WP + (16 * g - 1) * WP + c0,
                                                [(H * WP, 4), (WP, 18), (1, nw)]))
                                rhss.append(rt)
                            # disp residual enters PSUM via an identity matmul,
                            # so the epilogue is a single Act relu.
                            dt_ = pout2.tile([16, xw], bf16, tag="dtile", name="dtile")
                            nc.gpsimd.dma_start(
                                out=dt_[:],
                                in_=dap(dispb, (16 * g) * W + xo, [(W, 16), (1, xw)]))
                            ps = ppsum2.tile([16, 512], f32, tag="psc", name="psc")
                            for p_ in range(2):
                                for dx in range(3):
                                    nc.tensor.matmul(
                                        ps[:, :xw],
                                        sap(wcv, 72, (p_ * 3 + dx) * 16, [(1, 16)]),
                                        rhss[p_][:, dx:dx + xw],
                                        start=(p_ == 0 and dx == 0),
                                        stop=False)
                            nc.tensor.matmul(ps[:, :xw], idt[:], dt_[:],
                                             start=False, stop=True)
                            ot = pout2.tile([16, xw], f32, tag="cout", name="coutc")
                            if not in_scan and g % 2 == 0:
                                nc.vector.tensor_scalar(ot[:], ps[:, :xw], 0.0, None,
                                                        ALU.max)
                            else:
                                nc.scalar.activation(ot[:], ps[:, :xw], ACTF.Relu)
                            if in_scan:
                                weng = nc.gpsimd
                            else:
                                weng = nc.sync if g % 2 == 0 else nc.gpsimd
                            weng.dma_start(
                                out=dap(outp, (16 * g) * W + xo, [(W, 16), (1, xw)]),
                                in_=ot[:])

                    convc_part(0, 240, 0, 241, True)
                    convc_part(240, 240, 241, 240, True)
                    convc_part(480, 160, 481, 159, False)

    # Engine-sem update thinning: Tile increments each engine's sem on every
    # op, but only values that some wait references matter. Dropping the rest
    # (and renumbering waits to the kept-update count at the same producer op)
    # is semantically exact and shrinks the sem-update backlog.
    ENG_SEMS = ("DVE_44", "PE_44", "Activation_44", "Pool_44", "SP_44")
    insts_all = []
    for fn in nc.m.functions:
        for bb in fn.blocks:
            insts_all.extend(bb.instructions)
    waited = {sm: set() for sm in ENG_SEMS}
    for inst in insts_all:
        si = inst.sync_info
        if si and si.on_wait:
            for wt_ in si.on_wait:
                if wt_.ant_name in waited:
                    waited[wt_.ant_name].add(wt_.wait_value)
    cum = {sm: 0 for sm in ENG_SEMS}
    newcum = {sm: 0 for sm in ENG_SEMS}
    remap = {sm: {} for sm in ENG_SEMS}
    for inst in insts_all:
        si = inst.sync_info
        if not si:
            continue
        ups = si.on_update
        if ups:
            keep = []
            for u in ups:
                sm = u.ant_name
                if sm in cum:
                    cum[sm] += 1
                    if cum[sm] in waited[sm]:
                        newcum[sm] += 1
                        remap[sm][cum[sm]] = newcum[sm]
                        keep.append(u)
                    # else: drop this update
                else:
                    keep.append(u)
            if len(keep) != len(ups):
                si.on_update = keep
    for inst in insts_all:
        si = inst.sync_info
        if si and si.on_wait:
            ws = list(si.on_wait)
            ch = False
            for i_, wt_ in enumerate(ws):
                if wt_.ant_name in remap and wt_.wait_value in remap[wt_.ant_name]:
                    nv = remap[wt_.ant_name][wt_.wait_value]
                    if nv != wt_.wait_value:
                        wt_.wait_value = nv
                        ch = True
            if ch:
                si.on_wait = ws

    # walrus on this stack accepts at most one sync wait per instruction:
    # spill excess waits onto same-engine NOPs injected just before.
    nwn = [0]
    for fn in nc.m.functions:
        for bb in fn.blocks:
            il = bb.instructions
            i = 0
            while i < len(il):
                inst = il[i]
                si = inst.sync_info
                if si is not None and si.on_wait and len(si.on_wait) > 1:
                    waits = list(si.on_wait)
                    si.on_wait = waits[-1:]
                    for j in range(len(waits) - 1):
                        nwn[0] += 1
                        nop = mybir.InstNoOp(name=f"WS-{nwn[0]}", ins=[], outs=[])
                        nop.engine = inst.engine
                        nop.sync_info = mybir.SyncInfo(on_wait=[waits[j]], on_update=[])
                        nc.register_instruction(nop, overwrite=True)
                        il.insert(i, nop)
                        i += 1
                i += 1

    return nc


def _prep_inputs(inputs):
    w1, b1 = _fold_bn(inputs['w1'], inputs['bn1_g'], inputs['bn1_b'], inputs['bn1_m'], inputs['bn1_v'])
    w2, b2 = _fold_bn(inputs['w2'], inputs['bn2_g'], inputs['bn2_b'], inputs['bn2_m'], inputs['bn2_v'])
    w3, b3 = _fold_bn(inputs['w3'], inputs['bn3_g'], inputs['bn3_b'], inputs['bn3_m'], inputs['bn3_v'])

    w1k = _lhsT(w1, 8, 10, 3, 1).astype(BF)                       # [30, 384]
    w2k = _lhsT(w2, 6, 8, 16, 1).astype(BF)                       # [128, 288]
    w3k = _lhsT(w3, 6, 8, 16, 1).astype(BF)
    w4k = _lhsT(inputs['w4'].astype(np.float32), 5, 7, 16, 1).astype(BF)   # [112, 360]
    wdk = _lhsT_dx(inputs['wd'].astype(np.float32), 16, 18).astype(BF)     # [54, 128]
    wck = _lhsT(inputs['wc'].astype(np.float32), 16, 18, 4, 2).astype(BF)  # [72, 96]

    b1r = np.tile(b1, 8).reshape(128, 1).astype(np.float32)
    b2r = np.tile(b2, 6).reshape(96, 1).astype(np.float32)
    b3r = np.tile(b3, 6).reshape(96, 1).astype(np.float32)

    maps = []
    for b in range(8):
        maps.append({
            "img": _pad_rowi(inputs['leftImage'][b]),
            "dpad": _pad_rowi(inputs['disp'][b]),
            "dispb": inputs['disp'][b, 0].astype(BF),
            "w1k": w1k, "w2k": w2k, "w3k": w3k, "w4k": w4k, "wdk": wdk, "wck": wck,
            "b1v": b1r, "b2v": b2r, "b3v": b3r, "idv": np.eye(16, dtype=BF),
        })
    return maps


def kernel(**inputs):
    from concourse.bass_utils import run_bass_kernel_spmd

    if "nc" not in _CACHE:
        _CACHE["nc"] = _build()
    nc = _CACHE["nc"]
    maps = _prep_inputs(inputs)
    res = run_bass_kernel_spmd(nc, maps, core_ids=list(range(8)))
    out = np.stack([res.results[i]["out"] for i in range(8)])[:, None].astype(np.float32)
    return out
